# revision 10
# baseline (speedup 1.0000x reference)
"""Causal self-attention (B=4, T=2048, C=1024, H=16, D=64) on 8 TRN2 cores.

Sharding: core c handles batch b = c//2 and head-half hh = c%2 (8 heads).
Each core computes the qkv projection for its heads, causal attention, and
a partial output projection (its heads' rows of W_proj). Host sums the two
partials per batch and adds b_proj.

Per-core kernel (matmul operands in bf16 -> 1 cycle/row on the PE; all
accumulation in fp32 PSUM):
  phase 1: xT resident in SBUF; V = x@Wv + bv in [t, d] layout (+ ones
           column so PV also produces softmax row-sums); qkT = Wqk^T @ xT.
  phase 2: per head pair: S^T = K^T-tiles x Q (row-packed K=64 matmuls at
           partition bases 0/64), exp on ScalarE (1/sqrt(D) scale fused),
           causal by skipping upper-triangle s-tiles, narrowing diagonal
           tiles to their valid column range, and one [128,128] triangular
           mask multiply per diagonal tile; PV accumulation (M=65 with the
           row-sum column); normalization via DVE fast reciprocal + K=1
           fp32 broadcast matmul.
  phase 3: out = Y @ Wp from SBUF-resident Y^T.
"""

from contextlib import ExitStack

import ml_dtypes
import numpy as np

import concourse.bass as bass
import concourse.tile as tile
from concourse import bacc, mybir
from concourse.bass_utils import run_bass_kernel_spmd

F32 = mybir.dt.float32
DT = mybir.dt.bfloat16
NPDT = ml_dtypes.bfloat16
EXP = mybir.ActivationFunctionType.Exp

T = 2048        # tokens per core (one batch element)
C = 1024        # embed dim
H = 8           # local heads per core
D = 64          # head dim
P = 128
CT = C // P     # 8 contraction tiles over embed dim
QC = H * D      # 512 q/k/v channels per core
TJN = T // 512  # 4 t-tiles (free dim) for attention
SIN = T // P    # 16 s-tiles

TRACE = False   # set by test.py for profiling runs


def build_program():
    nc = bacc.Bacc("TRN2", target_bir_lowering=False, debug=False)
    xT = nc.dram_tensor("xT", [C, T], DT, kind="ExternalInput").ap()
    wqk = nc.dram_tensor("wqk", [C, 2 * QC], DT, kind="ExternalInput").ap()
    bqk = nc.dram_tensor("bqk", [2 * QC], F32, kind="ExternalInput").ap()
    wv = nc.dram_tensor("wv", [C, QC], DT, kind="ExternalInput").ap()
    bv = nc.dram_tensor("bv", [QC], DT, kind="ExternalInput").ap()
    wp = nc.dram_tensor("wp", [QC, C], DT, kind="ExternalInput").ap()
    trimask = nc.dram_tensor("trimask", [P, P], DT, kind="ExternalInput").ap()
    ones_in = nc.dram_tensor("ones", [P, P], DT, kind="ExternalInput").ap()
    onesf = nc.dram_tensor("onesf", [P, D], mybir.dt.float32r, kind="ExternalInput").ap()
    out = nc.dram_tensor("out", [T, C], F32, kind="ExternalOutput").ap()

    with tile.TileContext(nc) as tc, ExitStack() as persist:
        p_small = persist.enter_context(tc.tile_pool(name="small", bufs=1))
        bqk_sb = p_small.tile([P, CT], F32, tag="bqk")
        nc.sync.dma_start(bqk_sb, bqk.rearrange("(j p) -> p j", p=P))
        bv_sb = p_small.tile([1, QC], DT, tag="bv")
        nc.sync.dma_start(bv_sb, bv[None, :])
        ones_row = p_small.tile([1, P], DT, tag="ones_row")
        nc.sync.dma_start(ones_row, ones_in[0:1, :])
        ones64f = p_small.tile([P, D], mybir.dt.float32r, tag="ones64f")
        nc.sync.dma_start(ones64f, onesf)
        tri_sb = p_small.tile([P, P], DT, tag="tri")
        nc.sync.dma_start(tri_sb, trimask)

        # persistent across phases 1-2
        p_qkt = persist.enter_context(tc.tile_pool(name="qkt", bufs=1))
        p_va = persist.enter_context(tc.tile_pool(name="va", bufs=1))
        qkt = [p_qkt.tile([P, T], DT, tag=f"qkt{i}", name=f"qkt{i}") for i in range(CT)]
        va = [p_va.tile([P, H, D + 1], DT, tag=f"va{i}", name=f"va{i}") for i in range(SIN)]

        # ---------------- phase 1: projections ----------------
        with ExitStack() as ph1:
            p_xt = ph1.enter_context(tc.tile_pool(name="xt", bufs=1))
            ps1 = ph1.enter_context(tc.tile_pool(name="ps1", bufs=4, space="PSUM"))
            xt = [p_xt.tile([P, T], DT, tag=f"xt{j}", name=f"xt{j}") for j in range(CT)]
            for j in range(CT):
                nc.sync.dma_start(xt[j], xT[j * P:(j + 1) * P, :])

            # V projection: V[t, d] for all 8 heads at once (+bias via K=1 mm)
            with tc.tile_pool(name="wv", bufs=1) as p_wv:
                wv_sb = [p_wv.tile([P, QC], DT, tag=f"wv{j}", name=f"wv{j}") for j in range(CT)]
                for j in range(CT):
                    nc.sync.dma_start(wv_sb[j], wv[j * P:(j + 1) * P, :])
                for tt in range(SIN):
                    pv = ps1.tile([P, QC], F32, tag="ps1")
                    for j in range(CT):
                        nc.tensor.matmul(
                            pv, lhsT=xt[j][:, tt * P:(tt + 1) * P],
                            rhs=wv_sb[j], start=(j == 0), stop=False)
                    nc.tensor.matmul(pv, lhsT=ones_row, rhs=bv_sb,
                                     start=False, stop=True)
                    nc.vector.tensor_copy(out=va[tt][:, :, 0:D], in_=pv)
                    nc.sync.dma_start(va[tt][:, :, D:D + 1], ones_in[:, 0:H][:, :, None])

            # q/k projection: qkT[ch, t] = Wqk^T @ xT
            with tc.tile_pool(name="wqk", bufs=16) as p_wqk:
                for ch in range(CT):
                    wt = [p_wqk.tile([P, P], DT, tag="wqk", name="wqk") for _ in range(CT)]
                    for j in range(CT):
                        nc.sync.dma_start(
                            wt[j], wqk[j * P:(j + 1) * P, ch * P:(ch + 1) * P])
                    for tj in range(TJN):
                        pq = ps1.tile([P, 512], F32, tag="ps1")
                        for j in range(CT):
                            nc.tensor.matmul(
                                pq, lhsT=wt[j],
                                rhs=xt[j][:, tj * 512:(tj + 1) * 512],
                                start=(j == 0), stop=(j == CT - 1))
                        nc.vector.tensor_scalar_add(
                            out=qkt[ch][:, tj * 512:(tj + 1) * 512],
                            in0=pq, scalar1=bqk_sb[:, ch:ch + 1])

        # ---------------- phases 2+3 ----------------
        with ExitStack() as ph23:
            p_ysb = ph23.enter_context(tc.tile_pool(name="ysb", bufs=1))
            ysb = [p_ysb.tile([P, T], DT, tag=f"ysb{i}", name=f"ysb{i}")
                   for i in range(QC // P)]

            with ExitStack() as ph2:
                p_pt = ph2.enter_context(tc.tile_pool(name="pt", bufs=3))
                p_rcp = ph2.enter_context(tc.tile_pool(name="rcp", bufs=2))
                p_yn = ph2.enter_context(tc.tile_pool(name="yn", bufs=4))
                ps_s = ph2.enter_context(tc.tile_pool(name="ps_s", bufs=2, space="PSUM"))
                ps_y = ph2.enter_context(tc.tile_pool(name="ps_y", bufs=3, space="PSUM"))
                ps_r = ph2.enter_context(tc.tile_pool(name="ps_r", bufs=1, space="PSUM"))

                for hp in range(4):  # head pairs (local heads 2hp, 2hp+1)
                    qt, kt = qkt[hp], qkt[4 + hp]
                    for tj in range(TJN):
                        nsi = 4 * tj + 4
                        ya = ps_y.tile([D + 1, 512], F32, tag="ps_y")
                        yb = ps_y.tile([D + 1, 512], F32, tag="ps_y")
                        for si in range(nsi):
                            m = si - 4 * tj  # diagonal-band index (>=0 on diag)
                            o = max(m, 0) * P  # first valid column in this tj block
                            w = 512 - o
                            s = ps_s.tile([P, 1024], F32, tag="ps_s")
                            nc.tensor.matmul(
                                s[:, o:512], lhsT=kt[0:D, si * P:(si + 1) * P],
                                rhs=qt[0:D, tj * 512 + o:(tj + 1) * 512],
                                start=True, stop=True)
                            nc.tensor.matmul(
                                s[:, 512 + o:1024], lhsT=kt[D:P, si * P:(si + 1) * P],
                                rhs=qt[D:P, tj * 512 + o:(tj + 1) * 512],
                                start=True, stop=True)
                            pt = p_pt.tile([P, 1024], DT, tag="pt")
                            if m < 0:
                                nc.scalar.activation(pt, s, EXP, scale=0.125)
                            else:
                                nc.scalar.activation(pt[:, o:512], s[:, o:512],
                                                     EXP, scale=0.125)
                                nc.scalar.activation(pt[:, 512 + o:1024],
                                                     s[:, 512 + o:1024],
                                                     EXP, scale=0.125)
                                nc.vector.tensor_mul(
                                    pt[:, o:o + P], pt[:, o:o + P], tri_sb)
                                nc.vector.tensor_mul(
                                    pt[:, 512 + o:512 + o + P],
                                    pt[:, 512 + o:512 + o + P], tri_sb)
                            nc.tensor.matmul(
                                ya[:, o:512], lhsT=va[si][:, 2 * hp, :],
                                rhs=pt[:, o:512],
                                start=(si == 0), stop=(si == nsi - 1))
                            nc.tensor.matmul(
                                yb[:, o:512], lhsT=va[si][:, 2 * hp + 1, :],
                                rhs=pt[:, 512 + o:1024],
                                start=(si == 0), stop=(si == nsi - 1))
                        # normalize: rows 0-63 are Y, row 64 is the softmax sum
                        rcp = p_rcp.tile([P, 1024], mybir.dt.float32r, tag="rcp")
                        with nc.allow_low_precision(reason="elementwise recip"):
                            nc.vector.reciprocal(rcp[D:D + 1, 0:512], ya[D:D + 1, :])
                            nc.vector.reciprocal(rcp[D:D + 1, 512:1024], yb[D:D + 1, :])
                        ra = ps_r.tile([D, 512], F32, tag="ps_r")
                        nc.tensor.matmul(ra, lhsT=ones64f[D:D + 1, :],
                                         rhs=rcp[D:D + 1, 0:512],
                                         start=True, stop=True)
                        ra_sb = p_yn.tile([D, 512], F32, tag="r_sb", name="ra_sb")
                        nc.vector.tensor_copy(ra_sb, ra)
                        rb = ps_r.tile([D, 512], F32, tag="ps_r")
                        nc.tensor.matmul(rb, lhsT=ones64f[D:D + 1, :],
                                         rhs=rcp[D:D + 1, 512:1024],
                                         start=True, stop=True)
                        rb_sb = p_yn.tile([D, 512], F32, tag="r_sb", name="rb_sb")
                        nc.vector.tensor_copy(rb_sb, rb)
                        ts = slice(tj * 512, (tj + 1) * 512)
                        # even head: multiply straight into Y^T rows 0-63
                        nc.vector.tensor_mul(ysb[hp][0:D, ts], ya[0:D, :], ra_sb)
                        # odd head: stage (DVE lanes are partition-locked),
                        # DMA moves rows to 64-127
                        ynb = p_yn.tile([D, 512], DT, tag="yn")
                        nc.vector.tensor_mul(ynb, yb[0:D, :], rb_sb)
                        nc.sync.dma_start(ysb[hp][D:P, ts], ynb)

            # ---------------- phase 3: output projection ----------------
            with ExitStack() as ph3:
                p_wp = ph3.enter_context(tc.tile_pool(name="wp", bufs=1))
                p_o = ph3.enter_context(tc.tile_pool(name="o", bufs=3))
                ps3 = ph3.enter_context(tc.tile_pool(name="ps3", bufs=4, space="PSUM"))
                wpt = [p_wp.tile([P, C], DT, tag=f"wp{i}", name=f"wp{i}")
                       for i in range(QC // P)]
                for i in range(QC // P):
                    nc.sync.dma_start(wpt[i], wp[i * P:(i + 1) * P, :])
                for tt in range(SIN):
                    for co in range(C // 512):
                        po = ps3.tile([P, 512], F32, tag="ps3")
                        for i in range(QC // P):
                            nc.tensor.matmul(
                                po, lhsT=ysb[i][:, tt * P:(tt + 1) * P],
                                rhs=wpt[i][:, co * 512:(co + 1) * 512],
                                start=(i == 0), stop=(i == QC // P - 1))
                        ot = p_o.tile([P, 512], F32, tag="o")
                        nc.vector.tensor_copy(ot, po)
                        nc.sync.dma_start(
                            out[tt * P:(tt + 1) * P, co * 512:(co + 1) * 512], ot)

    nc.compile()
    return nc


_PROG = None


def _get_prog():
    global _PROG
    if _PROG is None:
        _PROG = build_program()
    return _PROG


_LAST_RESULT = {}


def kernel(x, W_attn, b_attn, W_proj, b_proj):
    x = np.asarray(x, np.float32)
    W_attn = np.asarray(W_attn, np.float32)
    b_attn = np.asarray(b_attn, np.float32)
    W_proj = np.asarray(W_proj, np.float32)
    b_proj = np.asarray(b_proj, np.float32)
    B = x.shape[0]
    nc = _get_prog()
    f = np.arange(P)[None, :]
    p = np.arange(P)[:, None]
    tri = (f >= p).astype(NPDT)
    cvt = lambda a: np.ascontiguousarray(a).astype(NPDT)
    in_maps = []
    for c in range(2 * B):
        b, hh = divmod(c, 2)
        sl = slice(hh * QC, hh * QC + QC)
        in_maps.append({
            "xT": cvt(x[b].T),
            "wqk": cvt(np.concatenate(
                [W_attn[:, sl], W_attn[:, C + hh * QC:C + hh * QC + QC]], axis=1)),
            "bqk": np.ascontiguousarray(np.concatenate(
                [b_attn[sl], b_attn[C + hh * QC:C + hh * QC + QC]])),
            "wv": cvt(W_attn[:, 2 * C + hh * QC:2 * C + hh * QC + QC]),
            "bv": cvt(b_attn[2 * C + hh * QC:2 * C + hh * QC + QC]),
            "wp": cvt(W_proj[hh * QC:hh * QC + QC, :]),
            "trimask": tri,
            "ones": np.ones((P, P), NPDT),
            "onesf": np.ones((P, D), np.float32),
        })
    res = run_bass_kernel_spmd(nc, in_maps, list(range(2 * B)), trace=TRACE)
    _LAST_RESULT["res"] = res
    out = np.empty((B, T, C), np.float32)
    for b in range(B):
        out[b] = res.results[2 * b]["out"] + res.results[2 * b + 1]["out"] + b_proj
    return out


# revision 12
# speedup vs baseline: 1.2090x; 1.2090x over previous
"""Causal self-attention (B=4, T=2048, C=1024, H=16, D=64) on 8 TRN2 cores.

Sharding: core c handles batch b = c//2 and head-half hh = c%2 (8 heads).
Each core computes the qkv projection for its heads, causal attention, and
a partial output projection (its heads' rows of W_proj). Host sums the two
partials per batch and adds b_proj.

Per-core kernel (matmul operands in bf16 -> 1 cycle/row on the PE; all
accumulation in fp32 PSUM):
  phase 1: xT resident in SBUF; V = x@Wv + bv in [t, d] layout (+ ones
           column so PV also produces softmax row-sums); qkT = Wqk^T @ xT.
  phase 2: per head pair: S^T = K^T-tiles x Q (row-packed K=64 matmuls at
           partition bases 0/64), exp on ScalarE (1/sqrt(D) scale fused),
           causal by skipping upper-triangle s-tiles, narrowing diagonal
           tiles to their valid column range, and one [128,128] triangular
           mask multiply per diagonal tile; PV accumulation (M=65 with the
           row-sum column); normalization via DVE fast reciprocal + K=1
           fp32 broadcast matmul.
  phase 3: out = Y @ Wp from SBUF-resident Y^T.
"""

from contextlib import ExitStack

import ml_dtypes
import numpy as np

import concourse.bass as bass
import concourse.tile as tile
from concourse import bacc, mybir
from concourse.bass_utils import run_bass_kernel_spmd

F32 = mybir.dt.float32
DT = mybir.dt.bfloat16
NPDT = ml_dtypes.bfloat16
EXP = mybir.ActivationFunctionType.Exp

T = 2048        # tokens per core (one batch element)
C = 1024        # embed dim
H = 8           # local heads per core
D = 64          # head dim
P = 128
CT = C // P     # 8 contraction tiles over embed dim
QC = H * D      # 512 q/k/v channels per core
TJN = T // 512  # 4 t-tiles (free dim) for attention
SIN = T // P    # 16 s-tiles

TRACE = False   # set by test.py for profiling runs


def build_program():
    nc = bacc.Bacc("TRN2", target_bir_lowering=False, debug=False)
    xT = nc.dram_tensor("xT", [C, T], DT, kind="ExternalInput").ap()
    wqk = nc.dram_tensor("wqk", [C, 2 * QC], DT, kind="ExternalInput").ap()
    bqk = nc.dram_tensor("bqk", [2 * QC], F32, kind="ExternalInput").ap()
    wv = nc.dram_tensor("wv", [C, QC], DT, kind="ExternalInput").ap()
    bv = nc.dram_tensor("bv", [QC], DT, kind="ExternalInput").ap()
    wp = nc.dram_tensor("wp", [QC, C], DT, kind="ExternalInput").ap()
    trimask = nc.dram_tensor("trimask", [P, P], DT, kind="ExternalInput").ap()
    ones_in = nc.dram_tensor("ones", [P, P], DT, kind="ExternalInput").ap()
    onesf = nc.dram_tensor("onesf", [P, D], mybir.dt.float32r, kind="ExternalInput").ap()
    out = nc.dram_tensor("out", [T, C], F32, kind="ExternalOutput").ap()

    with tile.TileContext(nc) as tc, ExitStack() as persist:
        p_small = persist.enter_context(tc.tile_pool(name="small", bufs=1))
        bqk_sb = p_small.tile([P, CT], F32, tag="bqk")
        nc.sync.dma_start(bqk_sb, bqk.rearrange("(j p) -> p j", p=P))
        bv_sb = p_small.tile([1, QC], DT, tag="bv")
        nc.sync.dma_start(bv_sb, bv[None, :])
        ones_row = p_small.tile([1, P], DT, tag="ones_row")
        nc.sync.dma_start(ones_row, ones_in[0:1, :])
        ones64f = p_small.tile([P, D], mybir.dt.float32r, tag="ones64f")
        nc.sync.dma_start(ones64f, onesf)
        tri_sb = p_small.tile([P, P], DT, tag="tri")
        nc.sync.dma_start(tri_sb, trimask)

        # persistent across phases 1-2
        p_qkt = persist.enter_context(tc.tile_pool(name="qkt", bufs=1))
        p_va = persist.enter_context(tc.tile_pool(name="va", bufs=1))
        qkt = [p_qkt.tile([P, T], DT, tag=f"qkt{i}", name=f"qkt{i}") for i in range(CT)]
        va = [p_va.tile([P, H, D + 1], DT, tag=f"va{i}", name=f"va{i}") for i in range(SIN)]

        # ---------------- phase 1: projections ----------------
        with ExitStack() as ph1:
            p_xt = ph1.enter_context(tc.tile_pool(name="xt", bufs=1))
            ps1 = ph1.enter_context(tc.tile_pool(name="ps1", bufs=4, space="PSUM"))
            xt = [p_xt.tile([P, T], DT, tag=f"xt{j}", name=f"xt{j}") for j in range(CT)]
            for j in range(CT):
                nc.sync.dma_start(xt[j], xT[j * P:(j + 1) * P, :])

            # V projection: V[t, d] for all 8 heads at once (+bias via K=1 mm)
            with tc.tile_pool(name="wv", bufs=1) as p_wv:
                wv_sb = [p_wv.tile([P, QC], DT, tag=f"wv{j}", name=f"wv{j}") for j in range(CT)]
                for j in range(CT):
                    nc.sync.dma_start(wv_sb[j], wv[j * P:(j + 1) * P, :])
                for tt in range(SIN):
                    pv = ps1.tile([P, QC], F32, tag="ps1")
                    for j in range(CT):
                        nc.tensor.matmul(
                            pv, lhsT=xt[j][:, tt * P:(tt + 1) * P],
                            rhs=wv_sb[j], start=(j == 0), stop=False)
                    nc.tensor.matmul(pv, lhsT=ones_row, rhs=bv_sb,
                                     start=False, stop=True)
                    nc.vector.tensor_copy(out=va[tt][:, :, 0:D], in_=pv)
                    nc.sync.dma_start(va[tt][:, :, D:D + 1], ones_in[:, 0:H][:, :, None])

            # q/k projection: qkT[ch, t] = Wqk^T @ xT
            with tc.tile_pool(name="wqk", bufs=16) as p_wqk:
                for ch in range(CT):
                    wt = [p_wqk.tile([P, P], DT, tag="wqk", name="wqk") for _ in range(CT)]
                    for j in range(CT):
                        nc.sync.dma_start(
                            wt[j], wqk[j * P:(j + 1) * P, ch * P:(ch + 1) * P])
                    for tj in range(TJN):
                        pq = ps1.tile([P, 512], F32, tag="ps1")
                        for j in range(CT):
                            nc.tensor.matmul(
                                pq, lhsT=wt[j],
                                rhs=xt[j][:, tj * 512:(tj + 1) * 512],
                                start=(j == 0), stop=(j == CT - 1))
                        nc.vector.tensor_scalar_add(
                            out=qkt[ch][:, tj * 512:(tj + 1) * 512],
                            in0=pq, scalar1=bqk_sb[:, ch:ch + 1])

        # ---------------- phases 2+3 ----------------
        with ExitStack() as ph23:
            p_ysb = ph23.enter_context(tc.tile_pool(name="ysb", bufs=1))
            ysb = [p_ysb.tile([P, T], DT, tag=f"ysb{i}", name=f"ysb{i}")
                   for i in range(QC // P)]

            with ExitStack() as ph2:
                p_pt = ph2.enter_context(tc.tile_pool(name="pt", bufs=3))
                p_sumr = ph2.enter_context(tc.tile_pool(name="sumr", bufs=3))
                p_scat = ph2.enter_context(tc.tile_pool(name="scat", bufs=3))
                p_rcpr = ph2.enter_context(tc.tile_pool(name="rcpr", bufs=6))
                p_yun = ph2.enter_context(tc.tile_pool(name="yun", bufs=12))
                p_rsb = ph2.enter_context(tc.tile_pool(name="rsb", bufs=3))
                p_yn = ph2.enter_context(tc.tile_pool(name="yn", bufs=3))
                p_dn = ph2.enter_context(tc.tile_pool(name="dn", bufs=4, space="DRAM"))
                ps_s = ph2.enter_context(tc.tile_pool(name="ps_s", bufs=2, space="PSUM"))
                ps_y = ph2.enter_context(tc.tile_pool(name="ps_y", bufs=2, space="PSUM"))
                ps_r = ph2.enter_context(tc.tile_pool(name="ps_r", bufs=2, space="PSUM"))

                def norm_batch(pending):
                    # deferred per-pair normalization: by now the recip rows
                    # are long done, so these matmuls never stall the PE FIFO
                    for hp, tj, rcp_row, yun_a, yun_b in pending:
                        ts = slice(tj * 512, (tj + 1) * 512)
                        for head, yun in ((0, yun_a), (1, yun_b)):
                            r = ps_r.tile([D, 512], F32, tag="ps_r", name="r")
                            nc.tensor.matmul(
                                r, lhsT=ones64f[D:D + 1, :],
                                rhs=rcp_row[D:D + 1, head * 512:(head + 1) * 512],
                                start=True, stop=True)
                            r_sb = p_rsb.tile([D, 512], F32, tag="rsb", name="r_sb")
                            nc.vector.tensor_copy(r_sb, r)
                            if head == 0:
                                nc.vector.tensor_mul(ysb[hp][0:D, ts], yun, r_sb)
                            else:
                                ynb = p_yn.tile([D, 512], DT, tag="yn", name="ynb")
                                nc.vector.tensor_mul(ynb, yun, r_sb)
                                nc.sync.dma_start(ysb[hp][D:P, ts], ynb)

                pending = []
                for hp in range(4):  # head pairs (local heads 2hp, 2hp+1)
                    qt, kt = qkt[hp], qkt[4 + hp]
                    for tj in range(TJN):
                        nsi = 4 * tj + 4
                        ya = ps_y.tile([D + 1, 512], F32, tag="ps_y")
                        yb = ps_y.tile([D + 1, 512], F32, tag="ps_y")
                        for si in range(nsi):
                            m = si - 4 * tj  # diagonal-band index (>=0 on diag)
                            o = max(m, 0) * P  # first valid column in this block
                            s = ps_s.tile([P, 1024], F32, tag="ps_s")
                            nc.tensor.matmul(
                                s[:, o:512], lhsT=kt[0:D, si * P:(si + 1) * P],
                                rhs=qt[0:D, tj * 512 + o:(tj + 1) * 512],
                                start=True, stop=True)
                            nc.tensor.matmul(
                                s[:, 512 + o:1024], lhsT=kt[D:P, si * P:(si + 1) * P],
                                rhs=qt[D:P, tj * 512 + o:(tj + 1) * 512],
                                start=True, stop=True)
                            pt = p_pt.tile([P, 1024], DT, tag="pt")
                            if m < 0:
                                nc.scalar.activation(pt, s, EXP, scale=0.125)
                            else:
                                nc.scalar.activation(pt[:, o:512], s[:, o:512],
                                                     EXP, scale=0.125)
                                nc.scalar.activation(pt[:, 512 + o:1024],
                                                     s[:, 512 + o:1024],
                                                     EXP, scale=0.125)
                                nc.vector.tensor_mul(
                                    pt[:, o:o + P], pt[:, o:o + P], tri_sb)
                                nc.vector.tensor_mul(
                                    pt[:, 512 + o:512 + o + P],
                                    pt[:, 512 + o:512 + o + P], tri_sb)
                            nc.tensor.matmul(
                                ya[:, o:512], lhsT=va[si][:, 2 * hp, :],
                                rhs=pt[:, o:512],
                                start=(si == 0), stop=(si == nsi - 1))
                            nc.tensor.matmul(
                                yb[:, o:512], lhsT=va[si][:, 2 * hp + 1, :],
                                rhs=pt[:, 512 + o:1024],
                                start=(si == 0), stop=(si == nsi - 1))
                        # release Y fast: copy unnormalized Y and the sums row
                        yun_a = p_yun.tile([D, 512], F32, tag="yun", name="yun_a")
                        yun_b = p_yun.tile([D, 512], F32, tag="yun", name="yun_b")
                        nc.vector.tensor_copy(yun_a, ya[0:D, :])
                        nc.vector.tensor_copy(yun_b, yb[0:D, :])
                        sumr = p_sumr.tile([P, 1024], F32, tag="sumr", name="sumr")
                        nc.vector.tensor_copy(sumr[D:D + 1, 0:512], ya[D:D + 1, :])
                        nc.vector.tensor_copy(sumr[D:D + 1, 512:1024], yb[D:D + 1, :])
                        # lane-parallel reciprocal: bounce through DRAM to a
                        # [128, 8] layout, recip on 128 lanes, bounce back
                        sums_d = p_dn.tile([1, 1024], F32, tag="sums_d", name="sums_d")
                        nc.sync.dma_start(sums_d, sumr[D:D + 1, :])
                        scat = p_scat.tile([P, 8], F32, tag="scat", name="scat")
                        nc.sync.dma_start(scat, sums_d.rearrange("1 (a b) -> a b", a=P))
                        scatr = p_scat.tile([P, 8], mybir.dt.float32r, tag="scatr",
                                            name="scatr")
                        with nc.allow_low_precision(reason="elementwise recip"):
                            nc.vector.reciprocal(scatr, scat)
                        rcp_d = p_dn.tile([1, 1024], mybir.dt.float32r, tag="rcp_d",
                                          name="rcp_d")
                        nc.sync.dma_start(rcp_d.rearrange("1 (a b) -> a b", a=P), scatr)
                        rcp_row = p_rcpr.tile([P, 1024], mybir.dt.float32r,
                                              tag="rcpr", name="rcp_row")
                        nc.sync.dma_start(rcp_row[D:D + 1, :], rcp_d)
                        pending.append((hp, tj, rcp_row, yun_a, yun_b))
                        if tj == 0 and pending[0][0] == hp - 1:
                            norm_batch(pending[:4])
                            pending = pending[4:]
                norm_batch(pending)

            # ---------------- phase 3: output projection ----------------
            with ExitStack() as ph3:
                p_wp = ph3.enter_context(tc.tile_pool(name="wp", bufs=1))
                p_o = ph3.enter_context(tc.tile_pool(name="o", bufs=3))
                ps3 = ph3.enter_context(tc.tile_pool(name="ps3", bufs=4, space="PSUM"))
                wpt = [p_wp.tile([P, C], DT, tag=f"wp{i}", name=f"wp{i}")
                       for i in range(QC // P)]
                for i in range(QC // P):
                    nc.sync.dma_start(wpt[i], wp[i * P:(i + 1) * P, :])
                for tt in range(SIN):
                    for co in range(C // 512):
                        po = ps3.tile([P, 512], F32, tag="ps3")
                        for i in range(QC // P):
                            nc.tensor.matmul(
                                po, lhsT=ysb[i][:, tt * P:(tt + 1) * P],
                                rhs=wpt[i][:, co * 512:(co + 1) * 512],
                                start=(i == 0), stop=(i == QC // P - 1))
                        ot = p_o.tile([P, 512], F32, tag="o")
                        nc.scalar.copy(ot, po)
                        nc.sync.dma_start(
                            out[tt * P:(tt + 1) * P, co * 512:(co + 1) * 512], ot)

    nc.compile()
    return nc


_PROG = None


def _get_prog():
    global _PROG
    if _PROG is None:
        _PROG = build_program()
    return _PROG


_LAST_RESULT = {}


def kernel(x, W_attn, b_attn, W_proj, b_proj):
    x = np.asarray(x, np.float32)
    W_attn = np.asarray(W_attn, np.float32)
    b_attn = np.asarray(b_attn, np.float32)
    W_proj = np.asarray(W_proj, np.float32)
    b_proj = np.asarray(b_proj, np.float32)
    B = x.shape[0]
    nc = _get_prog()
    f = np.arange(P)[None, :]
    p = np.arange(P)[:, None]
    tri = (f >= p).astype(NPDT)
    cvt = lambda a: np.ascontiguousarray(a).astype(NPDT)
    in_maps = []
    for c in range(2 * B):
        b, hh = divmod(c, 2)
        sl = slice(hh * QC, hh * QC + QC)
        in_maps.append({
            "xT": cvt(x[b].T),
            "wqk": cvt(np.concatenate(
                [W_attn[:, sl], W_attn[:, C + hh * QC:C + hh * QC + QC]], axis=1)),
            "bqk": np.ascontiguousarray(np.concatenate(
                [b_attn[sl], b_attn[C + hh * QC:C + hh * QC + QC]])),
            "wv": cvt(W_attn[:, 2 * C + hh * QC:2 * C + hh * QC + QC]),
            "bv": cvt(b_attn[2 * C + hh * QC:2 * C + hh * QC + QC]),
            "wp": cvt(W_proj[hh * QC:hh * QC + QC, :]),
            "trimask": tri,
            "ones": np.ones((P, P), NPDT),
            "onesf": np.ones((P, D), np.float32),
        })
    res = run_bass_kernel_spmd(nc, in_maps, list(range(2 * B)), trace=TRACE)
    _LAST_RESULT["res"] = res
    out = np.empty((B, T, C), np.float32)
    for b in range(B):
        out[b] = res.results[2 * b]["out"] + res.results[2 * b + 1]["out"] + b_proj
    return out


# revision 13
# speedup vs baseline: 1.2184x; 1.0078x over previous
"""Causal self-attention (B=4, T=2048, C=1024, H=16, D=64) on 8 TRN2 cores.

Sharding: core c handles batch b = c//2 and head-half hh = c%2 (8 heads).
Each core computes the qkv projection for its heads, causal attention, and
a partial output projection (its heads' rows of W_proj). Host sums the two
partials per batch and adds b_proj.

Per-core kernel (matmul operands in bf16 -> 1 cycle/row on the PE; all
accumulation in fp32 PSUM):
  phase 1: xT resident in SBUF; V = x@Wv + bv in [t, d] layout (+ ones
           column so PV also produces softmax row-sums); qkT = Wqk^T @ xT.
  phase 2: per head pair: S^T = K^T-tiles x Q (row-packed K=64 matmuls at
           partition bases 0/64), exp on ScalarE (1/sqrt(D) scale fused),
           causal by skipping upper-triangle s-tiles, narrowing diagonal
           tiles to their valid column range, and one [128,128] triangular
           mask multiply per diagonal tile; PV accumulation (M=65 with the
           row-sum column); normalization via DVE fast reciprocal + K=1
           fp32 broadcast matmul.
  phase 3: out = Y @ Wp from SBUF-resident Y^T.
"""

from contextlib import ExitStack

import ml_dtypes
import numpy as np

import concourse.bass as bass
import concourse.tile as tile
from concourse import bacc, mybir
from concourse.bass_utils import run_bass_kernel_spmd

F32 = mybir.dt.float32
DT = mybir.dt.bfloat16
NPDT = ml_dtypes.bfloat16
EXP = mybir.ActivationFunctionType.Exp

T = 2048        # tokens per core (one batch element)
C = 1024        # embed dim
H = 8           # local heads per core
D = 64          # head dim
P = 128
CT = C // P     # 8 contraction tiles over embed dim
QC = H * D      # 512 q/k/v channels per core
TJN = T // 512  # 4 t-tiles (free dim) for attention
SIN = T // P    # 16 s-tiles

TRACE = False   # set by test.py for profiling runs


def build_program():
    nc = bacc.Bacc("TRN2", target_bir_lowering=False, debug=False)
    xT = nc.dram_tensor("xT", [C, T], DT, kind="ExternalInput").ap()
    wqk = nc.dram_tensor("wqk", [C, 2 * QC], DT, kind="ExternalInput").ap()
    bqk = nc.dram_tensor("bqk", [2 * QC], F32, kind="ExternalInput").ap()
    wv = nc.dram_tensor("wv", [C, QC], DT, kind="ExternalInput").ap()
    bv = nc.dram_tensor("bv", [QC], DT, kind="ExternalInput").ap()
    wp = nc.dram_tensor("wp", [QC, C], DT, kind="ExternalInput").ap()
    trimask = nc.dram_tensor("trimask", [P, P], DT, kind="ExternalInput").ap()
    ones_in = nc.dram_tensor("ones", [P, P], DT, kind="ExternalInput").ap()
    onesf = nc.dram_tensor("onesf", [P, D], mybir.dt.float32r, kind="ExternalInput").ap()
    out = nc.dram_tensor("out", [T, C], F32, kind="ExternalOutput").ap()

    with tile.TileContext(nc) as tc, ExitStack() as persist:
        p_small = persist.enter_context(tc.tile_pool(name="small", bufs=1))
        bqk_sb = p_small.tile([P, CT], F32, tag="bqk")
        nc.sync.dma_start(bqk_sb, bqk.rearrange("(j p) -> p j", p=P))
        bv_sb = p_small.tile([1, QC], DT, tag="bv")
        nc.sync.dma_start(bv_sb, bv[None, :])
        ones_row = p_small.tile([1, P], DT, tag="ones_row")
        nc.sync.dma_start(ones_row, ones_in[0:1, :])
        ones64f = p_small.tile([P, D], mybir.dt.float32r, tag="ones64f")
        nc.sync.dma_start(ones64f, onesf)
        tri_sb = p_small.tile([P, P], DT, tag="tri")
        nc.sync.dma_start(tri_sb, trimask)

        # persistent across phases 1-2
        p_qkt = persist.enter_context(tc.tile_pool(name="qkt", bufs=1))
        p_va = persist.enter_context(tc.tile_pool(name="va", bufs=1))
        qkt = [p_qkt.tile([P, T], DT, tag=f"qkt{i}", name=f"qkt{i}") for i in range(CT)]
        va = [p_va.tile([P, H, D + 1], DT, tag=f"va{i}", name=f"va{i}") for i in range(SIN)]

        # ---------------- phase 1: projections ----------------
        with ExitStack() as ph1:
            p_xt = ph1.enter_context(tc.tile_pool(name="xt", bufs=1))
            ps1 = ph1.enter_context(tc.tile_pool(name="ps1", bufs=4, space="PSUM"))
            xt = [p_xt.tile([P, T], DT, tag=f"xt{j}", name=f"xt{j}") for j in range(CT)]
            for j in range(CT):
                nc.sync.dma_start(xt[j][:, 0:T // 2], xT[j * P:(j + 1) * P, 0:T // 2])
            for j in range(CT):
                nc.sync.dma_start(xt[j][:, T // 2:T], xT[j * P:(j + 1) * P, T // 2:T])

            # V projection: V[t, d] for all 8 heads at once (+bias via K=1 mm)
            with tc.tile_pool(name="wv", bufs=1) as p_wv:
                wv_sb = [p_wv.tile([P, QC], DT, tag=f"wv{j}", name=f"wv{j}") for j in range(CT)]
                for j in range(CT):
                    nc.sync.dma_start(wv_sb[j], wv[j * P:(j + 1) * P, :])
                for tt in range(SIN):
                    pv = ps1.tile([P, QC], F32, tag="ps1")
                    for j in range(CT):
                        nc.tensor.matmul(
                            pv, lhsT=xt[j][:, tt * P:(tt + 1) * P],
                            rhs=wv_sb[j], start=(j == 0), stop=False)
                    nc.tensor.matmul(pv, lhsT=ones_row, rhs=bv_sb,
                                     start=False, stop=True)
                    nc.vector.tensor_copy(out=va[tt][:, :, 0:D], in_=pv)
                    nc.sync.dma_start(va[tt][:, :, D:D + 1], ones_in[:, 0:H][:, :, None])

            # q/k projection: qkT[ch, t] = Wqk^T @ xT
            with tc.tile_pool(name="wqk", bufs=16) as p_wqk:
                for ch in (0, 4, 1, 5, 2, 6, 3, 7):
                    wt = [p_wqk.tile([P, P], DT, tag="wqk", name="wqk") for _ in range(CT)]
                    for j in range(CT):
                        nc.sync.dma_start(
                            wt[j], wqk[j * P:(j + 1) * P, ch * P:(ch + 1) * P])
                    for tj in range(TJN):
                        pq = ps1.tile([P, 512], F32, tag="ps1")
                        for j in range(CT):
                            nc.tensor.matmul(
                                pq, lhsT=wt[j],
                                rhs=xt[j][:, tj * 512:(tj + 1) * 512],
                                start=(j == 0), stop=(j == CT - 1))
                        nc.vector.tensor_scalar_add(
                            out=qkt[ch][:, tj * 512:(tj + 1) * 512],
                            in0=pq, scalar1=bqk_sb[:, ch:ch + 1])

        # ------- phases 2+3, interleaved: proj trails attention by one tj -------
        with ExitStack() as ph23:
            p_ysb = ph23.enter_context(tc.tile_pool(name="ysb", bufs=1))
            ysb = [p_ysb.tile([P, T], DT, tag=f"ysb{i}", name=f"ysb{i}")
                   for i in range(QC // P)]
            p_wp = ph23.enter_context(tc.tile_pool(name="wp", bufs=1))
            wpt = [p_wp.tile([P, C], DT, tag=f"wp{i}", name=f"wp{i}")
                   for i in range(QC // P)]
            for i in range(QC // P):
                nc.sync.dma_start(wpt[i], wp[i * P:(i + 1) * P, :])
            p_pt = ph23.enter_context(tc.tile_pool(name="pt", bufs=3))
            p_sumr = ph23.enter_context(tc.tile_pool(name="sumr", bufs=3))
            p_scat = ph23.enter_context(tc.tile_pool(name="scat", bufs=3))
            p_rcpr = ph23.enter_context(tc.tile_pool(name="rcpr", bufs=4))
            p_yun = ph23.enter_context(tc.tile_pool(name="yun", bufs=6))
            p_rsb = ph23.enter_context(tc.tile_pool(name="rsb", bufs=3))
            p_yn = ph23.enter_context(tc.tile_pool(name="yn", bufs=3))
            p_o = ph23.enter_context(tc.tile_pool(name="o", bufs=4))
            p_dn = ph23.enter_context(tc.tile_pool(name="dn", bufs=4, space="DRAM"))
            ps_s = ph23.enter_context(tc.tile_pool(name="ps_s", bufs=2, space="PSUM"))
            ps_y = ph23.enter_context(tc.tile_pool(name="ps_y", bufs=2, space="PSUM"))
            # R (normalization broadcast) and proj output share this pool
            ps_r = ph23.enter_context(tc.tile_pool(name="ps_r", bufs=2, space="PSUM"))

            def norm_batch(hp, tj, rcp_row, yun_a, yun_b):
                ts = slice(tj * 512, (tj + 1) * 512)
                for head, yun in ((0, yun_a), (1, yun_b)):
                    r = ps_r.tile([P, 512], F32, tag="ps_r", name="r")
                    nc.tensor.matmul(
                        r[0:D, :], lhsT=ones64f[D:D + 1, :],
                        rhs=rcp_row[D:D + 1, head * 512:(head + 1) * 512],
                        start=True, stop=True)
                    r_sb = p_rsb.tile([D, 512], F32, tag="rsb", name="r_sb")
                    nc.vector.tensor_copy(r_sb, r[0:D, :])
                    if head == 0:
                        nc.vector.tensor_mul(ysb[hp][0:D, ts], yun, r_sb)
                    else:
                        ynb = p_yn.tile([D, 512], DT, tag="yn", name="ynb")
                        nc.vector.tensor_mul(ynb, yun, r_sb)
                        nc.sync.dma_start(ysb[hp][D:P, ts], ynb)

            def proj_group(tj):
                for tt in range(4 * tj, 4 * tj + 4):
                    for co in range(C // 512):
                        po = ps_r.tile([P, 512], F32, tag="ps_r", name="po")
                        for i in range(QC // P):
                            nc.tensor.matmul(
                                po, lhsT=ysb[i][:, tt * P:(tt + 1) * P],
                                rhs=wpt[i][:, co * 512:(co + 1) * 512],
                                start=(i == 0), stop=(i == QC // P - 1))
                        ot = p_o.tile([P, 512], F32, tag="o", name="ot")
                        if tt % 2 == 0:
                            nc.vector.tensor_copy(ot, po)
                        else:
                            nc.scalar.copy(ot, po)
                        nc.sync.dma_start(
                            out[tt * P:(tt + 1) * P, co * 512:(co + 1) * 512], ot)

            pending = []
            for tj in range(TJN):
                for hp in range(4):  # head pairs (local heads 2hp, 2hp+1)
                    qt, kt = qkt[hp], qkt[4 + hp]
                    nsi = 4 * tj + 4
                    ya = ps_y.tile([D + 1, 512], F32, tag="ps_y")
                    yb = ps_y.tile([D + 1, 512], F32, tag="ps_y")
                    for si in range(nsi):
                        m = si - 4 * tj  # diagonal-band index (>=0 on diag)
                        o = max(m, 0) * P  # first valid column in this block
                        s = ps_s.tile([P, 1024], F32, tag="ps_s")
                        nc.tensor.matmul(
                            s[:, o:512], lhsT=kt[0:D, si * P:(si + 1) * P],
                            rhs=qt[0:D, tj * 512 + o:(tj + 1) * 512],
                            start=True, stop=True)
                        nc.tensor.matmul(
                            s[:, 512 + o:1024], lhsT=kt[D:P, si * P:(si + 1) * P],
                            rhs=qt[D:P, tj * 512 + o:(tj + 1) * 512],
                            start=True, stop=True)
                        pt = p_pt.tile([P, 1024], DT, tag="pt")
                        if m < 0:
                            nc.scalar.activation(pt, s, EXP, scale=0.125)
                        else:
                            # one strided call covers both heads' valid range
                            pt2 = pt.rearrange("p (h w) -> p h w", h=2)
                            s2 = s.rearrange("p (h w) -> p h w", h=2)
                            nc.scalar.activation(pt2[:, :, o:512], s2[:, :, o:512],
                                                 EXP, scale=0.125)
                            nc.vector.tensor_tensor(
                                pt2[:, :, o:o + P], pt2[:, :, o:o + P],
                                tri_sb[:, None, :].to_broadcast((P, 2, P)),
                                mybir.AluOpType.mult)
                        nc.tensor.matmul(
                            ya[:, o:512], lhsT=va[si][:, 2 * hp, :],
                            rhs=pt[:, o:512],
                            start=(si == 0), stop=(si == nsi - 1))
                        nc.tensor.matmul(
                            yb[:, o:512], lhsT=va[si][:, 2 * hp + 1, :],
                            rhs=pt[:, 512 + o:1024],
                            start=(si == 0), stop=(si == nsi - 1))
                    # release Y fast: copy unnormalized Y and the sums row
                    yun_a = p_yun.tile([D, 512], F32, tag="yun", name="yun_a")
                    yun_b = p_yun.tile([D, 512], F32, tag="yun", name="yun_b")
                    nc.vector.tensor_copy(yun_a, ya[0:D, :])
                    nc.vector.tensor_copy(yun_b, yb[0:D, :])
                    sumr = p_sumr.tile([P, 1024], F32, tag="sumr", name="sumr")
                    nc.vector.tensor_copy(sumr[D:D + 1, 0:512], ya[D:D + 1, :])
                    nc.vector.tensor_copy(sumr[D:D + 1, 512:1024], yb[D:D + 1, :])
                    # lane-parallel reciprocal via a DRAM bounce to [128, 8]
                    sums_d = p_dn.tile([1, 1024], F32, tag="sums_d", name="sums_d")
                    nc.sync.dma_start(sums_d, sumr[D:D + 1, :])
                    scat = p_scat.tile([P, 8], F32, tag="scat", name="scat")
                    nc.sync.dma_start(scat, sums_d.rearrange("1 (a b) -> a b", a=P))
                    scatr = p_scat.tile([P, 8], mybir.dt.float32r, tag="scatr",
                                        name="scatr")
                    with nc.allow_low_precision(reason="elementwise recip"):
                        nc.vector.reciprocal(scatr, scat)
                    rcp_d = p_dn.tile([1, 1024], mybir.dt.float32r, tag="rcp_d",
                                      name="rcp_d")
                    nc.sync.dma_start(rcp_d.rearrange("1 (a b) -> a b", a=P), scatr)
                    rcp_row = p_rcpr.tile([P, 1024], mybir.dt.float32r,
                                          tag="rcpr", name="rcp_row")
                    nc.sync.dma_start(rcp_row[D:D + 1, :], rcp_d)
                    pending.append((hp, tj, rcp_row, yun_a, yun_b))
                    if len(pending) >= 2:
                        norm_batch(*pending.pop(0))
                if tj >= 1:
                    proj_group(tj - 1)
            while pending:
                norm_batch(*pending.pop(0))
            proj_group(TJN - 1)

    nc.compile()
    return nc


_PROG = None


def _get_prog():
    global _PROG
    if _PROG is None:
        _PROG = build_program()
    return _PROG


_LAST_RESULT = {}


def kernel(x, W_attn, b_attn, W_proj, b_proj):
    x = np.asarray(x, np.float32)
    W_attn = np.asarray(W_attn, np.float32)
    b_attn = np.asarray(b_attn, np.float32)
    W_proj = np.asarray(W_proj, np.float32)
    b_proj = np.asarray(b_proj, np.float32)
    B = x.shape[0]
    nc = _get_prog()
    f = np.arange(P)[None, :]
    p = np.arange(P)[:, None]
    tri = (f >= p).astype(NPDT)
    cvt = lambda a: np.ascontiguousarray(a).astype(NPDT)
    in_maps = []
    for c in range(2 * B):
        b, hh = divmod(c, 2)
        sl = slice(hh * QC, hh * QC + QC)
        in_maps.append({
            "xT": cvt(x[b].T),
            "wqk": cvt(np.concatenate(
                [W_attn[:, sl], W_attn[:, C + hh * QC:C + hh * QC + QC]], axis=1)),
            "bqk": np.ascontiguousarray(np.concatenate(
                [b_attn[sl], b_attn[C + hh * QC:C + hh * QC + QC]])),
            "wv": cvt(W_attn[:, 2 * C + hh * QC:2 * C + hh * QC + QC]),
            "bv": cvt(b_attn[2 * C + hh * QC:2 * C + hh * QC + QC]),
            "wp": cvt(W_proj[hh * QC:hh * QC + QC, :]),
            "trimask": tri,
            "ones": np.ones((P, P), NPDT),
            "onesf": np.ones((P, D), np.float32),
        })
    res = run_bass_kernel_spmd(nc, in_maps, list(range(2 * B)), trace=TRACE)
    _LAST_RESULT["res"] = res
    out = np.empty((B, T, C), np.float32)
    for b in range(B):
        out[b] = res.results[2 * b]["out"] + res.results[2 * b + 1]["out"] + b_proj
    return out


# revision 15
# speedup vs baseline: 1.2456x; 1.0222x over previous
"""Causal self-attention (B=4, T=2048, C=1024, H=16, D=64) on 8 TRN2 cores.

Sharding: core c handles batch b = c//2 and head-half hh = c%2 (8 heads).
Each core computes the qkv projection for its heads, causal attention, and
a partial output projection (its heads' rows of W_proj). Host sums the two
partials per batch and adds b_proj.

Per-core kernel (matmul operands in bf16 -> 1 cycle/row on the PE; all
accumulation in fp32 PSUM):
  phase 1: xT resident in SBUF; V = x@Wv + bv in [t, d] layout (+ ones
           column so PV also produces softmax row-sums); qkT = Wqk^T @ xT.
  phase 2: per head pair: S^T = K^T-tiles x Q (row-packed K=64 matmuls at
           partition bases 0/64), exp on ScalarE (1/sqrt(D) scale fused),
           causal by skipping upper-triangle s-tiles, narrowing diagonal
           tiles to their valid column range, and one [128,128] triangular
           mask multiply per diagonal tile; PV accumulation (M=65 with the
           row-sum column); normalization via DVE fast reciprocal + K=1
           fp32 broadcast matmul.
  phase 3: out = Y @ Wp from SBUF-resident Y^T.
"""

from contextlib import ExitStack

import ml_dtypes
import numpy as np

import concourse.bass as bass
import concourse.tile as tile
from concourse import bacc, mybir
from concourse.bass_utils import run_bass_kernel_spmd

F32 = mybir.dt.float32
DT = mybir.dt.bfloat16
NPDT = ml_dtypes.bfloat16
EXP = mybir.ActivationFunctionType.Exp

T = 2048        # tokens per core (one batch element)
C = 1024        # embed dim
H = 8           # local heads per core
D = 64          # head dim
P = 128
CT = C // P     # 8 contraction tiles over embed dim
QC = H * D      # 512 q/k/v channels per core
TJN = T // 512  # 4 t-tiles (free dim) for attention
SIN = T // P    # 16 s-tiles

TRACE = False   # set by test.py for profiling runs


def build_program():
    nc = bacc.Bacc("TRN2", target_bir_lowering=False, debug=False)
    xT = nc.dram_tensor("xT", [C, T], DT, kind="ExternalInput").ap()
    wqk = nc.dram_tensor("wqk", [C, 2 * QC], DT, kind="ExternalInput").ap()
    bqk = nc.dram_tensor("bqk", [2 * QC], F32, kind="ExternalInput").ap()
    wv = nc.dram_tensor("wv", [C, QC], DT, kind="ExternalInput").ap()
    bv = nc.dram_tensor("bv", [QC], DT, kind="ExternalInput").ap()
    wp = nc.dram_tensor("wp", [QC, C], DT, kind="ExternalInput").ap()
    trimask = nc.dram_tensor("trimask", [P, P], DT, kind="ExternalInput").ap()
    ones_in = nc.dram_tensor("ones", [P, P], DT, kind="ExternalInput").ap()
    onesf = nc.dram_tensor("onesf", [P, D], mybir.dt.float32r, kind="ExternalInput").ap()
    out = nc.dram_tensor("out", [T, C], F32, kind="ExternalOutput").ap()

    with tile.TileContext(nc) as tc, ExitStack() as persist:
        p_small = persist.enter_context(tc.tile_pool(name="small", bufs=1))
        bqk_sb = p_small.tile([P, CT], F32, tag="bqk")
        nc.sync.dma_start(bqk_sb, bqk.rearrange("(j p) -> p j", p=P))
        bv_sb = p_small.tile([1, QC], DT, tag="bv")
        nc.sync.dma_start(bv_sb, bv[None, :])
        ones_row = p_small.tile([1, P], DT, tag="ones_row")
        nc.sync.dma_start(ones_row, ones_in[0:1, :])
        ones64f = p_small.tile([P, D], mybir.dt.float32r, tag="ones64f")
        nc.sync.dma_start(ones64f, onesf)
        tri_sb = p_small.tile([P, P], DT, tag="tri")
        nc.sync.dma_start(tri_sb, trimask)

        # persistent across phases 1-2
        p_qkt = persist.enter_context(tc.tile_pool(name="qkt", bufs=1))
        p_va = persist.enter_context(tc.tile_pool(name="va", bufs=1))
        qkt = [p_qkt.tile([P, T], DT, tag=f"qkt{i}", name=f"qkt{i}") for i in range(CT)]
        va = [p_va.tile([P, H, D + 1], DT, tag=f"va{i}", name=f"va{i}") for i in range(SIN)]

        # ---------------- phase 1: projections ----------------
        with ExitStack() as ph1:
            p_xt = ph1.enter_context(tc.tile_pool(name="xt", bufs=1))
            ps1 = ph1.enter_context(tc.tile_pool(name="ps1", bufs=4, space="PSUM"))
            xt = [p_xt.tile([P, T], DT, tag=f"xt{j}", name=f"xt{j}") for j in range(CT)]
            for j in range(CT):
                nc.sync.dma_start(xt[j][:, 0:T // 2], xT[j * P:(j + 1) * P, 0:T // 2])
            for j in range(CT):
                nc.sync.dma_start(xt[j][:, T // 2:T], xT[j * P:(j + 1) * P, T // 2:T])

            # V projection: V[t, d] for all 8 heads at once (+bias via K=1 mm)
            with tc.tile_pool(name="wv", bufs=1) as p_wv:
                wv_sb = [p_wv.tile([P, QC], DT, tag=f"wv{j}", name=f"wv{j}") for j in range(CT)]
                for j in range(CT):
                    nc.sync.dma_start(wv_sb[j], wv[j * P:(j + 1) * P, :])
                for tt in range(SIN):
                    pv = ps1.tile([P, QC], F32, tag="ps1")
                    for j in range(CT):
                        nc.tensor.matmul(
                            pv, lhsT=xt[j][:, tt * P:(tt + 1) * P],
                            rhs=wv_sb[j], start=(j == 0), stop=False)
                    nc.tensor.matmul(pv, lhsT=ones_row, rhs=bv_sb,
                                     start=False, stop=True)
                    nc.vector.tensor_copy(out=va[tt][:, :, 0:D], in_=pv)
                    nc.sync.dma_start(va[tt][:, :, D:D + 1], ones_in[:, 0:H][:, :, None])

            # q/k projection: qkT[ch, t] = Wqk^T @ xT
            with tc.tile_pool(name="wqk", bufs=16) as p_wqk:
                for ch in (0, 4, 1, 5, 2, 6, 3, 7):
                    wt = [p_wqk.tile([P, P], DT, tag="wqk", name="wqk") for _ in range(CT)]
                    for j in range(CT):
                        nc.sync.dma_start(
                            wt[j], wqk[j * P:(j + 1) * P, ch * P:(ch + 1) * P])
                    for tj in range(TJN):
                        pq = ps1.tile([P, 512], F32, tag="ps1")
                        for j in range(CT):
                            nc.tensor.matmul(
                                pq, lhsT=wt[j],
                                rhs=xt[j][:, tj * 512:(tj + 1) * 512],
                                start=(j == 0), stop=(j == CT - 1))
                        nc.vector.tensor_scalar_add(
                            out=qkt[ch][:, tj * 512:(tj + 1) * 512],
                            in0=pq, scalar1=bqk_sb[:, ch:ch + 1])

        # ------- phases 2+3, interleaved: proj trails attention by one tj -------
        with ExitStack() as ph23:
            p_ysb = ph23.enter_context(tc.tile_pool(name="ysb", bufs=1))
            ysb = [p_ysb.tile([P, T], DT, tag=f"ysb{i}", name=f"ysb{i}")
                   for i in range(QC // P)]
            p_wp = ph23.enter_context(tc.tile_pool(name="wp", bufs=1))
            wpt = [p_wp.tile([P, C], DT, tag=f"wp{i}", name=f"wp{i}")
                   for i in range(QC // P)]
            for i in range(QC // P):
                nc.sync.dma_start(wpt[i], wp[i * P:(i + 1) * P, :])
            p_pt = ph23.enter_context(tc.tile_pool(name="pt", bufs=3))
            p_sumr = ph23.enter_context(tc.tile_pool(name="sumr", bufs=3))
            p_scat = ph23.enter_context(tc.tile_pool(name="scat", bufs=3))
            p_rcpr = ph23.enter_context(tc.tile_pool(name="rcpr", bufs=4))
            p_yun = ph23.enter_context(tc.tile_pool(name="yun", bufs=6))
            p_rsb = ph23.enter_context(tc.tile_pool(name="rsb", bufs=3))
            p_yn = ph23.enter_context(tc.tile_pool(name="yn", bufs=3))
            p_o = ph23.enter_context(tc.tile_pool(name="o", bufs=4))
            p_dn = ph23.enter_context(tc.tile_pool(name="dn", bufs=4, space="DRAM"))
            ps_s = ph23.enter_context(tc.tile_pool(name="ps_s", bufs=2, space="PSUM"))
            ps_y = ph23.enter_context(tc.tile_pool(name="ps_y", bufs=2, space="PSUM"))
            # R (normalization broadcast) and proj output share this pool
            ps_r = ph23.enter_context(tc.tile_pool(name="ps_r", bufs=2, space="PSUM"))

            def norm_batch(hp, tj, rcp_row, yun_a, yun_b):
                ts = slice(tj * 512, (tj + 1) * 512)
                for head, yun in ((0, yun_a), (1, yun_b)):
                    r = ps_r.tile([P, 512], F32, tag="ps_r", name="r")
                    nc.tensor.matmul(
                        r[0:D, :], lhsT=ones64f[D:D + 1, :],
                        rhs=rcp_row[D:D + 1, head * 512:(head + 1) * 512],
                        start=True, stop=True)
                    r_sb = p_rsb.tile([D, 512], F32, tag="rsb", name="r_sb")
                    nc.vector.tensor_copy(r_sb, r[0:D, :])
                    if head == 0:
                        nc.vector.tensor_mul(ysb[hp][0:D, ts], yun, r_sb)
                    else:
                        ynb = p_yn.tile([D, 512], DT, tag="yn", name="ynb")
                        nc.vector.tensor_mul(ynb, yun, r_sb)
                        nc.sync.dma_start(ysb[hp][D:P, ts], ynb)

            def proj_tile(tt, co):
                po = ps_r.tile([P, 512], F32, tag="ps_r", name="po")
                for i in range(QC // P):
                    nc.tensor.matmul(
                        po, lhsT=ysb[i][:, tt * P:(tt + 1) * P],
                        rhs=wpt[i][:, co * 512:(co + 1) * 512],
                        start=(i == 0), stop=(i == QC // P - 1))
                ot = p_o.tile([P, 512], F32, tag="o", name="ot")
                if tt % 2 == 0:
                    nc.vector.tensor_copy(ot, po)
                else:
                    nc.scalar.copy(ot, po)
                nc.sync.dma_start(
                    out[tt * P:(tt + 1) * P, co * 512:(co + 1) * 512], ot)

            pending = []
            projq = []  # ready-to-run proj tiles, popped between si iterations
            sictr = 0
            for tj in range(TJN):
                for hp in range(4):  # head pairs (local heads 2hp, 2hp+1)
                    if hp == 1 and tj >= 1:
                        projq += [(tt, co) for tt in range(4 * (tj - 1), 4 * tj)
                                  for co in range(C // 512)]
                    qt, kt = qkt[hp], qkt[4 + hp]
                    nsi = 4 * tj + 4
                    ya = ps_y.tile([D + 1, 512], F32, tag="ps_y")
                    yb = ps_y.tile([D + 1, 512], F32, tag="ps_y")
                    for si in range(nsi):
                        m = si - 4 * tj  # diagonal-band index (>=0 on diag)
                        o = max(m, 0) * P  # first valid column in this block
                        s = ps_s.tile([P, 1024], F32, tag="ps_s")
                        nc.tensor.matmul(
                            s[:, o:512], lhsT=kt[0:D, si * P:(si + 1) * P],
                            rhs=qt[0:D, tj * 512 + o:(tj + 1) * 512],
                            start=True, stop=True)
                        nc.tensor.matmul(
                            s[:, 512 + o:1024], lhsT=kt[D:P, si * P:(si + 1) * P],
                            rhs=qt[D:P, tj * 512 + o:(tj + 1) * 512],
                            start=True, stop=True)
                        pt = p_pt.tile([P, 1024], DT, tag="pt")
                        if m < 0:
                            nc.scalar.activation(pt, s, EXP, scale=0.125)
                        else:
                            # one strided call covers both heads' valid range
                            pt2 = pt.rearrange("p (h w) -> p h w", h=2)
                            s2 = s.rearrange("p (h w) -> p h w", h=2)
                            nc.scalar.activation(pt2[:, :, o:512], s2[:, :, o:512],
                                                 EXP, scale=0.125)
                            nc.vector.tensor_tensor(
                                pt2[:, :, o:o + P], pt2[:, :, o:o + P],
                                tri_sb[:, None, :].to_broadcast((P, 2, P)),
                                mybir.AluOpType.mult)
                        nc.tensor.matmul(
                            ya[:, o:512], lhsT=va[si][:, 2 * hp, :],
                            rhs=pt[:, o:512],
                            start=(si == 0), stop=(si == nsi - 1))
                        nc.tensor.matmul(
                            yb[:, o:512], lhsT=va[si][:, 2 * hp + 1, :],
                            rhs=pt[:, 512 + o:1024],
                            start=(si == 0), stop=(si == nsi - 1))
                        sictr += 1
                        if projq and sictr % 3 == 0:
                            proj_tile(*projq.pop(0))
                    # release Y fast: copy unnormalized Y and the sums row
                    yun_a = p_yun.tile([D, 512], F32, tag="yun", name="yun_a")
                    yun_b = p_yun.tile([D, 512], F32, tag="yun", name="yun_b")
                    nc.vector.tensor_copy(yun_a, ya[0:D, :])
                    nc.vector.tensor_copy(yun_b, yb[0:D, :])
                    sumr = p_sumr.tile([P, 1024], F32, tag="sumr", name="sumr")
                    nc.vector.tensor_copy(sumr[D:D + 1, 0:512], ya[D:D + 1, :])
                    nc.vector.tensor_copy(sumr[D:D + 1, 512:1024], yb[D:D + 1, :])
                    # lane-parallel reciprocal via a DRAM bounce to [128, 8]
                    sums_d = p_dn.tile([1, 1024], F32, tag="sums_d", name="sums_d")
                    nc.sync.dma_start(sums_d, sumr[D:D + 1, :])
                    scat = p_scat.tile([P, 8], F32, tag="scat", name="scat")
                    nc.sync.dma_start(scat, sums_d.rearrange("1 (a b) -> a b", a=P))
                    scatr = p_scat.tile([P, 8], mybir.dt.float32r, tag="scatr",
                                        name="scatr")
                    with nc.allow_low_precision(reason="elementwise recip"):
                        nc.vector.reciprocal(scatr, scat)
                    rcp_d = p_dn.tile([1, 1024], mybir.dt.float32r, tag="rcp_d",
                                      name="rcp_d")
                    nc.sync.dma_start(rcp_d.rearrange("1 (a b) -> a b", a=P), scatr)
                    rcp_row = p_rcpr.tile([P, 1024], mybir.dt.float32r,
                                          tag="rcpr", name="rcp_row")
                    nc.sync.dma_start(rcp_row[D:D + 1, :], rcp_d)
                    pending.append((hp, tj, rcp_row, yun_a, yun_b))
                    if len(pending) >= 2:
                        norm_batch(*pending.pop(0))
            while pending:
                norm_batch(*pending.pop(0))
            while projq:
                proj_tile(*projq.pop(0))
            for tt in range(4 * (TJN - 1), 4 * TJN):
                for co in range(C // 512):
                    proj_tile(tt, co)

    nc.compile()
    return nc


_PROG = None


def _get_prog():
    global _PROG
    if _PROG is None:
        _PROG = build_program()
    return _PROG


_LAST_RESULT = {}


def kernel(x, W_attn, b_attn, W_proj, b_proj):
    x = np.asarray(x, np.float32)
    W_attn = np.asarray(W_attn, np.float32)
    b_attn = np.asarray(b_attn, np.float32)
    W_proj = np.asarray(W_proj, np.float32)
    b_proj = np.asarray(b_proj, np.float32)
    B = x.shape[0]
    nc = _get_prog()
    f = np.arange(P)[None, :]
    p = np.arange(P)[:, None]
    tri = (f >= p).astype(NPDT)
    cvt = lambda a: np.ascontiguousarray(a).astype(NPDT)
    in_maps = []
    for c in range(2 * B):
        b, hh = divmod(c, 2)
        sl = slice(hh * QC, hh * QC + QC)
        in_maps.append({
            "xT": cvt(x[b].T),
            "wqk": cvt(np.concatenate(
                [W_attn[:, sl], W_attn[:, C + hh * QC:C + hh * QC + QC]], axis=1)),
            "bqk": np.ascontiguousarray(np.concatenate(
                [b_attn[sl], b_attn[C + hh * QC:C + hh * QC + QC]])),
            "wv": cvt(W_attn[:, 2 * C + hh * QC:2 * C + hh * QC + QC]),
            "bv": cvt(b_attn[2 * C + hh * QC:2 * C + hh * QC + QC]),
            "wp": cvt(W_proj[hh * QC:hh * QC + QC, :]),
            "trimask": tri,
            "ones": np.ones((P, P), NPDT),
            "onesf": np.ones((P, D), np.float32),
        })
    res = run_bass_kernel_spmd(nc, in_maps, list(range(2 * B)), trace=TRACE)
    _LAST_RESULT["res"] = res
    out = np.empty((B, T, C), np.float32)
    for b in range(B):
        out[b] = res.results[2 * b]["out"] + res.results[2 * b + 1]["out"] + b_proj
    return out


# revision 16
# speedup vs baseline: 1.3200x; 1.0597x over previous
"""Causal self-attention (B=4, T=2048, C=1024, H=16, D=64) on 8 TRN2 cores.

Sharding: core c handles batch b = c//2 and head-half hh = c%2 (8 heads).
Each core computes the qkv projection for its heads, causal attention, and
a partial output projection (its heads' rows of W_proj). Host sums the two
partials per batch and adds b_proj.

Per-core kernel (matmul operands in bf16 -> 1 cycle/row on the PE; all
accumulation in fp32 PSUM):
  phase 1: xT resident in SBUF; V = x@Wv + bv in [t, d] layout (+ ones
           column so PV also produces softmax row-sums); qkT = Wqk^T @ xT.
  phase 2: per head pair: S^T = K^T-tiles x Q (row-packed K=64 matmuls at
           partition bases 0/64), exp on ScalarE (1/sqrt(D) scale fused),
           causal by skipping upper-triangle s-tiles, narrowing diagonal
           tiles to their valid column range, and one [128,128] triangular
           mask multiply per diagonal tile; PV accumulation (M=65 with the
           row-sum column); normalization via DVE fast reciprocal + K=1
           fp32 broadcast matmul.
  phase 3: out = Y @ Wp from SBUF-resident Y^T.
"""

from contextlib import ExitStack

import ml_dtypes
import numpy as np

import concourse.bass as bass
import concourse.tile as tile
from concourse import bacc, mybir
from concourse.bass_utils import run_bass_kernel_spmd

F32 = mybir.dt.float32
DT = mybir.dt.bfloat16
NPDT = ml_dtypes.bfloat16
EXP = mybir.ActivationFunctionType.Exp

T = 2048        # tokens per core (one batch element)
C = 1024        # embed dim
H = 8           # local heads per core
D = 64          # head dim
P = 128
CT = C // P     # 8 contraction tiles over embed dim
QC = H * D      # 512 q/k/v channels per core
TJN = T // 512  # 4 t-tiles (free dim) for attention
SIN = T // P    # 16 s-tiles

TRACE = False   # set by test.py for profiling runs


def build_program():
    nc = bacc.Bacc("TRN2", target_bir_lowering=False, debug=False)
    xT = nc.dram_tensor("xT", [C, T], DT, kind="ExternalInput").ap()
    wqk = nc.dram_tensor("wqk", [C, 2 * QC], DT, kind="ExternalInput").ap()
    bqk = nc.dram_tensor("bqk", [2 * QC], F32, kind="ExternalInput").ap()
    wv = nc.dram_tensor("wv", [C, QC], DT, kind="ExternalInput").ap()
    bv = nc.dram_tensor("bv", [QC], DT, kind="ExternalInput").ap()
    wp = nc.dram_tensor("wp", [QC, C], DT, kind="ExternalInput").ap()
    trimask = nc.dram_tensor("trimask", [P, P], DT, kind="ExternalInput").ap()
    ones_in = nc.dram_tensor("ones", [P, P], DT, kind="ExternalInput").ap()
    onesf = nc.dram_tensor("onesf", [P, D], mybir.dt.float32r, kind="ExternalInput").ap()
    out = nc.dram_tensor("out", [T, C], F32, kind="ExternalOutput").ap()

    with tile.TileContext(nc) as tc, ExitStack() as persist:
        p_small = persist.enter_context(tc.tile_pool(name="small", bufs=1))
        bqk_sb = p_small.tile([P, CT], F32, tag="bqk")
        nc.sync.dma_start(bqk_sb, bqk.rearrange("(j p) -> p j", p=P))
        bv_sb = p_small.tile([1, QC], DT, tag="bv")
        nc.sync.dma_start(bv_sb, bv[None, :])
        ones_row = p_small.tile([1, P], DT, tag="ones_row")
        nc.sync.dma_start(ones_row, ones_in[0:1, :])
        ones64f = p_small.tile([P, D], mybir.dt.float32r, tag="ones64f")
        nc.sync.dma_start(ones64f, onesf)
        tri_sb = p_small.tile([P, P], DT, tag="tri")
        nc.sync.dma_start(tri_sb, trimask)

        # persistent across phases 1-2
        p_qkt = persist.enter_context(tc.tile_pool(name="qkt", bufs=1))
        p_va = persist.enter_context(tc.tile_pool(name="va", bufs=1))
        qkt = [p_qkt.tile([P, T], DT, tag=f"qkt{i}", name=f"qkt{i}") for i in range(CT)]
        va = [p_va.tile([P, H, D + 1], DT, tag=f"va{i}", name=f"va{i}") for i in range(SIN)]

        # ---------------- merged phases ----------------
        with ExitStack() as ph:
            p_xt = ph.enter_context(tc.tile_pool(name="xt", bufs=1))
            p_wqk = ph.enter_context(tc.tile_pool(name="wqk", bufs=16))
            xt = [p_xt.tile([P, T], DT, tag=f"xt{j}", name=f"xt{j}") for j in range(CT)]
            for j in range(CT):
                nc.sync.dma_start(xt[j][:, 0:T // 2], xT[j * P:(j + 1) * P, 0:T // 2])
            for j in range(CT):
                nc.sync.dma_start(xt[j][:, T // 2:T], xT[j * P:(j + 1) * P, T // 2:T])

            p_ysb = ph.enter_context(tc.tile_pool(name="ysb", bufs=1))
            ysb = [p_ysb.tile([P, T], DT, tag=f"ysb{i}", name=f"ysb{i}")
                   for i in range(QC // P)]
            p_wp = ph.enter_context(tc.tile_pool(name="wp", bufs=1))
            wpt = [p_wp.tile([P, C], DT, tag=f"wp{i}", name=f"wp{i}")
                   for i in range(QC // P)]
            for i in range(QC // P):
                nc.sync.dma_start(wpt[i], wp[i * P:(i + 1) * P, :])
            p_pt = ph.enter_context(tc.tile_pool(name="pt", bufs=4))
            p_sumr = ph.enter_context(tc.tile_pool(name="sumr", bufs=3))
            p_scat = ph.enter_context(tc.tile_pool(name="scat", bufs=3))
            p_rcpr = ph.enter_context(tc.tile_pool(name="rcpr", bufs=4))
            p_yun = ph.enter_context(tc.tile_pool(name="yun", bufs=6))
            p_rsb = ph.enter_context(tc.tile_pool(name="rsb", bufs=3))
            p_yn = ph.enter_context(tc.tile_pool(name="yn", bufs=3))
            p_o = ph.enter_context(tc.tile_pool(name="o", bufs=4))
            p_dn = ph.enter_context(tc.tile_pool(name="dn", bufs=4, space="DRAM"))
            ps_s = ph.enter_context(tc.tile_pool(name="ps_s", bufs=2, space="PSUM"))
            ps_y = ph.enter_context(tc.tile_pool(name="ps_y", bufs=2, space="PSUM"))
            # shared by v-proj, qk-proj, normalization R, and proj outputs
            ps_r = ph.enter_context(tc.tile_pool(name="ps_r", bufs=2, space="PSUM"))

            # V projection: V[t, d] for all 8 heads at once (+bias via K=1 mm)
            with tc.tile_pool(name="wv", bufs=1) as p_wv:
                wv_sb = [p_wv.tile([P, QC], DT, tag=f"wv{j}", name=f"wv{j}") for j in range(CT)]
                for j in range(CT):
                    nc.sync.dma_start(wv_sb[j], wv[j * P:(j + 1) * P, :])
                for tt in range(SIN):
                    pv = ps_r.tile([P, QC], F32, tag="ps_r", name="pv")
                    for j in range(CT):
                        nc.tensor.matmul(
                            pv, lhsT=xt[j][:, tt * P:(tt + 1) * P],
                            rhs=wv_sb[j], start=(j == 0), stop=False)
                    nc.tensor.matmul(pv, lhsT=ones_row, rhs=bv_sb,
                                     start=False, stop=True)
                    nc.vector.tensor_copy(out=va[tt][:, :, 0:D], in_=pv)
                    nc.sync.dma_start(va[tt][:, :, D:D + 1], ones_in[:, 0:H][:, :, None])

            def qkproj(ch):
                wt = [p_wqk.tile([P, P], DT, tag="wqk", name="wqk") for _ in range(CT)]
                for j in range(CT):
                    nc.sync.dma_start(
                        wt[j], wqk[j * P:(j + 1) * P, ch * P:(ch + 1) * P])
                for tjc in range(TJN):
                    pq = ps_r.tile([P, 512], F32, tag="ps_r", name="pq")
                    for j in range(CT):
                        nc.tensor.matmul(
                            pq, lhsT=wt[j],
                            rhs=xt[j][:, tjc * 512:(tjc + 1) * 512],
                            start=(j == 0), stop=(j == CT - 1))
                    nc.vector.tensor_scalar_add(
                        out=qkt[ch][:, tjc * 512:(tjc + 1) * 512],
                        in0=pq, scalar1=bqk_sb[:, ch:ch + 1])

            def norm_batch(hp, tj, rcp_row, yun_a, yun_b):
                ts = slice(tj * 512, (tj + 1) * 512)
                for head, yun in ((0, yun_a), (1, yun_b)):
                    r = ps_r.tile([P, 512], F32, tag="ps_r", name="r")
                    nc.tensor.matmul(
                        r[0:D, :], lhsT=ones64f[D:D + 1, :],
                        rhs=rcp_row[D:D + 1, head * 512:(head + 1) * 512],
                        start=True, stop=True)
                    r_sb = p_rsb.tile([D, 512], F32, tag="rsb", name="r_sb")
                    nc.vector.tensor_copy(r_sb, r[0:D, :])
                    if head == 0:
                        nc.vector.tensor_mul(ysb[hp][0:D, ts], yun, r_sb)
                    else:
                        ynb = p_yn.tile([D, 512], DT, tag="yn", name="ynb")
                        nc.vector.tensor_mul(ynb, yun, r_sb)
                        nc.sync.dma_start(ysb[hp][D:P, ts], ynb)

            def proj_tile(tt, co):
                po = ps_r.tile([P, 512], F32, tag="ps_r", name="po")
                for i in range(QC // P):
                    nc.tensor.matmul(
                        po, lhsT=ysb[i][:, tt * P:(tt + 1) * P],
                        rhs=wpt[i][:, co * 512:(co + 1) * 512],
                        start=(i == 0), stop=(i == QC // P - 1))
                ot = p_o.tile([P, 512], F32, tag="o", name="ot")
                if tt % 2 == 0:
                    nc.vector.tensor_copy(ot, po)
                else:
                    nc.scalar.copy(ot, po)
                nc.sync.dma_start(
                    out[tt * P:(tt + 1) * P, co * 512:(co + 1) * 512], ot)

            pending = []
            projq = []  # ready-to-run proj tiles, popped between si iterations
            sictr = 0
            for tj in range(TJN):
                for hp in range(4):  # head pairs (local heads 2hp, 2hp+1)
                    if tj == 0:
                        qkproj(hp)
                        qkproj(4 + hp)
                    if hp == 1 and tj >= 1:
                        projq += [(tt, co) for tt in range(4 * (tj - 1), 4 * tj)
                                  for co in range(C // 512)]
                    qt, kt = qkt[hp], qkt[4 + hp]
                    nsi = 4 * tj + 4
                    ya = ps_y.tile([D + 1, 512], F32, tag="ps_y")
                    yb = ps_y.tile([D + 1, 512], F32, tag="ps_y")
                    for si in range(nsi):
                        m = si - 4 * tj  # diagonal-band index (>=0 on diag)
                        o = max(m, 0) * P  # first valid column in this block
                        s = ps_s.tile([P, 1024], F32, tag="ps_s")
                        nc.tensor.matmul(
                            s[:, o:512], lhsT=kt[0:D, si * P:(si + 1) * P],
                            rhs=qt[0:D, tj * 512 + o:(tj + 1) * 512],
                            start=True, stop=True)
                        nc.tensor.matmul(
                            s[:, 512 + o:1024], lhsT=kt[D:P, si * P:(si + 1) * P],
                            rhs=qt[D:P, tj * 512 + o:(tj + 1) * 512],
                            start=True, stop=True)
                        pt = p_pt.tile([P, 1024], DT, tag="pt")
                        if m < 0:
                            nc.scalar.activation(pt, s, EXP, scale=0.125)
                        else:
                            # one strided call covers both heads' valid range
                            pt2 = pt.rearrange("p (h w) -> p h w", h=2)
                            s2 = s.rearrange("p (h w) -> p h w", h=2)
                            nc.scalar.activation(pt2[:, :, o:512], s2[:, :, o:512],
                                                 EXP, scale=0.125)
                            nc.vector.tensor_tensor(
                                pt2[:, :, o:o + P], pt2[:, :, o:o + P],
                                tri_sb[:, None, :].to_broadcast((P, 2, P)),
                                mybir.AluOpType.mult)
                        nc.tensor.matmul(
                            ya[:, o:512], lhsT=va[si][:, 2 * hp, :],
                            rhs=pt[:, o:512],
                            start=(si == 0), stop=(si == nsi - 1))
                        nc.tensor.matmul(
                            yb[:, o:512], lhsT=va[si][:, 2 * hp + 1, :],
                            rhs=pt[:, 512 + o:1024],
                            start=(si == 0), stop=(si == nsi - 1))
                        sictr += 1
                        if projq and sictr % 3 == 0:
                            proj_tile(*projq.pop(0))
                    # release Y fast: copy unnormalized Y and the sums row
                    yun_a = p_yun.tile([D, 512], F32, tag="yun", name="yun_a")
                    yun_b = p_yun.tile([D, 512], F32, tag="yun", name="yun_b")
                    nc.vector.tensor_copy(yun_a, ya[0:D, :])
                    nc.vector.tensor_copy(yun_b, yb[0:D, :])
                    sumr = p_sumr.tile([P, 1024], F32, tag="sumr", name="sumr")
                    nc.vector.tensor_copy(sumr[D:D + 1, 0:512], ya[D:D + 1, :])
                    nc.vector.tensor_copy(sumr[D:D + 1, 512:1024], yb[D:D + 1, :])
                    # lane-parallel reciprocal via a DRAM bounce to [128, 8]
                    sums_d = p_dn.tile([1, 1024], F32, tag="sums_d", name="sums_d")
                    nc.sync.dma_start(sums_d, sumr[D:D + 1, :])
                    scat = p_scat.tile([P, 8], F32, tag="scat", name="scat")
                    nc.sync.dma_start(scat, sums_d.rearrange("1 (a b) -> a b", a=P))
                    scatr = p_scat.tile([P, 8], mybir.dt.float32r, tag="scatr",
                                        name="scatr")
                    with nc.allow_low_precision(reason="elementwise recip"):
                        nc.vector.reciprocal(scatr, scat)
                    rcp_d = p_dn.tile([1, 1024], mybir.dt.float32r, tag="rcp_d",
                                      name="rcp_d")
                    nc.sync.dma_start(rcp_d.rearrange("1 (a b) -> a b", a=P), scatr)
                    rcp_row = p_rcpr.tile([P, 1024], mybir.dt.float32r,
                                          tag="rcpr", name="rcp_row")
                    nc.sync.dma_start(rcp_row[D:D + 1, :], rcp_d)
                    pending.append((hp, tj, rcp_row, yun_a, yun_b))
                    if len(pending) >= 2:
                        norm_batch(*pending.pop(0))
            while pending:
                norm_batch(*pending.pop(0))
            while projq:
                proj_tile(*projq.pop(0))
            for tt in range(4 * (TJN - 1), 4 * TJN):
                for co in range(C // 512):
                    proj_tile(tt, co)

    nc.compile()
    return nc


_PROG = None


def _get_prog():
    global _PROG
    if _PROG is None:
        _PROG = build_program()
    return _PROG


_LAST_RESULT = {}


def kernel(x, W_attn, b_attn, W_proj, b_proj):
    x = np.asarray(x, np.float32)
    W_attn = np.asarray(W_attn, np.float32)
    b_attn = np.asarray(b_attn, np.float32)
    W_proj = np.asarray(W_proj, np.float32)
    b_proj = np.asarray(b_proj, np.float32)
    B = x.shape[0]
    nc = _get_prog()
    f = np.arange(P)[None, :]
    p = np.arange(P)[:, None]
    tri = (f >= p).astype(NPDT)
    cvt = lambda a: np.ascontiguousarray(a).astype(NPDT)
    in_maps = []
    for c in range(2 * B):
        b, hh = divmod(c, 2)
        sl = slice(hh * QC, hh * QC + QC)
        in_maps.append({
            "xT": cvt(x[b].T),
            "wqk": cvt(np.concatenate(
                [W_attn[:, sl], W_attn[:, C + hh * QC:C + hh * QC + QC]], axis=1)),
            "bqk": np.ascontiguousarray(np.concatenate(
                [b_attn[sl], b_attn[C + hh * QC:C + hh * QC + QC]])),
            "wv": cvt(W_attn[:, 2 * C + hh * QC:2 * C + hh * QC + QC]),
            "bv": cvt(b_attn[2 * C + hh * QC:2 * C + hh * QC + QC]),
            "wp": cvt(W_proj[hh * QC:hh * QC + QC, :]),
            "trimask": tri,
            "ones": np.ones((P, P), NPDT),
            "onesf": np.ones((P, D), np.float32),
        })
    res = run_bass_kernel_spmd(nc, in_maps, list(range(2 * B)), trace=TRACE)
    _LAST_RESULT["res"] = res
    out = np.empty((B, T, C), np.float32)
    for b in range(B):
        out[b] = res.results[2 * b]["out"] + res.results[2 * b + 1]["out"] + b_proj
    return out


# revision 18
# speedup vs baseline: 1.3208x; 1.0006x over previous
"""Causal self-attention (B=4, T=2048, C=1024, H=16, D=64) on 8 TRN2 cores.

Sharding: core c handles batch b = c//2 and head-half hh = c%2 (8 heads).
Each core computes the qkv projection for its heads, causal attention, and
a partial output projection (its heads' rows of W_proj). Host sums the two
partials per batch and adds b_proj.

Per-core kernel (matmul operands in bf16 -> 1 cycle/row on the PE; all
accumulation in fp32 PSUM):
  phase 1: xT resident in SBUF; V = x@Wv + bv in [t, d] layout (+ ones
           column so PV also produces softmax row-sums); qkT = Wqk^T @ xT.
  phase 2: per head pair: S^T = K^T-tiles x Q (row-packed K=64 matmuls at
           partition bases 0/64), exp on ScalarE (1/sqrt(D) scale fused),
           causal by skipping upper-triangle s-tiles, narrowing diagonal
           tiles to their valid column range, and one [128,128] triangular
           mask multiply per diagonal tile; PV accumulation (M=65 with the
           row-sum column); normalization via DVE fast reciprocal + K=1
           fp32 broadcast matmul.
  phase 3: out = Y @ Wp from SBUF-resident Y^T.
"""

from contextlib import ExitStack

import ml_dtypes
import numpy as np

import concourse.bass as bass
import concourse.tile as tile
from concourse import bacc, mybir
from concourse.bass_utils import run_bass_kernel_spmd

F32 = mybir.dt.float32
DT = mybir.dt.bfloat16
NPDT = ml_dtypes.bfloat16
EXP = mybir.ActivationFunctionType.Exp

T = 2048        # tokens per core (one batch element)
C = 1024        # embed dim
H = 8           # local heads per core
D = 64          # head dim
P = 128
CT = C // P     # 8 contraction tiles over embed dim
QC = H * D      # 512 q/k/v channels per core
TJN = T // 512  # 4 t-tiles (free dim) for attention
SIN = T // P    # 16 s-tiles

TRACE = False   # set by test.py for profiling runs


def build_program():
    nc = bacc.Bacc("TRN2", target_bir_lowering=False, debug=False)
    xT = nc.dram_tensor("xT", [C, T], DT, kind="ExternalInput").ap()
    wqk = nc.dram_tensor("wqk", [C, 2 * QC], DT, kind="ExternalInput").ap()
    bqk = nc.dram_tensor("bqk", [2 * QC], F32, kind="ExternalInput").ap()
    wv = nc.dram_tensor("wv", [C, QC], DT, kind="ExternalInput").ap()
    bv = nc.dram_tensor("bv", [QC], DT, kind="ExternalInput").ap()
    wp = nc.dram_tensor("wp", [QC, C], DT, kind="ExternalInput").ap()
    trimask = nc.dram_tensor("trimask", [P, P], DT, kind="ExternalInput").ap()
    ones_in = nc.dram_tensor("ones", [P, P], DT, kind="ExternalInput").ap()
    onesf = nc.dram_tensor("onesf", [P, D], mybir.dt.float32r, kind="ExternalInput").ap()
    out = nc.dram_tensor("out", [T, C], F32, kind="ExternalOutput").ap()

    with tile.TileContext(nc) as tc, ExitStack() as persist:
        p_small = persist.enter_context(tc.tile_pool(name="small", bufs=1))
        bqk_sb = p_small.tile([P, CT], F32, tag="bqk")
        nc.sync.dma_start(bqk_sb, bqk.rearrange("(j p) -> p j", p=P))
        bv_sb = p_small.tile([1, QC], DT, tag="bv")
        nc.sync.dma_start(bv_sb, bv[None, :])
        ones_row = p_small.tile([1, P], DT, tag="ones_row")
        nc.sync.dma_start(ones_row, ones_in[0:1, :])
        ones64f = p_small.tile([P, D], mybir.dt.float32r, tag="ones64f")
        nc.sync.dma_start(ones64f, onesf)
        tri_sb = p_small.tile([P, P], DT, tag="tri")
        nc.sync.dma_start(tri_sb, trimask)

        # persistent across phases 1-2
        p_qkt = persist.enter_context(tc.tile_pool(name="qkt", bufs=1))
        p_va = persist.enter_context(tc.tile_pool(name="va", bufs=1))
        qkt = [p_qkt.tile([P, T], DT, tag=f"qkt{i}", name=f"qkt{i}") for i in range(CT)]
        va = [p_va.tile([P, H, D + 1], DT, tag=f"va{i}", name=f"va{i}") for i in range(SIN)]

        # ---------------- merged phases ----------------
        with ExitStack() as ph:
            p_xt = ph.enter_context(tc.tile_pool(name="xt", bufs=1))
            p_wqk = ph.enter_context(tc.tile_pool(name="wqk", bufs=16))
            xt = [p_xt.tile([P, T], DT, tag=f"xt{j}", name=f"xt{j}") for j in range(CT)]
            for j in range(CT):
                nc.sync.dma_start(xt[j][:, 0:T // 2], xT[j * P:(j + 1) * P, 0:T // 2])
            for j in range(CT):
                nc.sync.dma_start(xt[j][:, T // 2:T], xT[j * P:(j + 1) * P, T // 2:T])

            p_ysb = ph.enter_context(tc.tile_pool(name="ysb", bufs=1))
            ysb = [p_ysb.tile([P, T], DT, tag=f"ysb{i}", name=f"ysb{i}")
                   for i in range(QC // P)]
            p_wp = ph.enter_context(tc.tile_pool(name="wp", bufs=1))
            wpt = [p_wp.tile([P, C], DT, tag=f"wp{i}", name=f"wp{i}")
                   for i in range(QC // P)]
            for i in range(QC // P):
                nc.sync.dma_start(wpt[i], wp[i * P:(i + 1) * P, :])
            p_pt = ph.enter_context(tc.tile_pool(name="pt", bufs=4))
            p_sumr = ph.enter_context(tc.tile_pool(name="sumr", bufs=3))
            p_scat = ph.enter_context(tc.tile_pool(name="scat", bufs=3))
            p_rcpr = ph.enter_context(tc.tile_pool(name="rcpr", bufs=4))
            p_yun = ph.enter_context(tc.tile_pool(name="yun", bufs=6))
            p_rsb = ph.enter_context(tc.tile_pool(name="rsb", bufs=3))
            p_yn = ph.enter_context(tc.tile_pool(name="yn", bufs=3))
            p_o = ph.enter_context(tc.tile_pool(name="o", bufs=4))
            p_dn = ph.enter_context(tc.tile_pool(name="dn", bufs=4, space="DRAM"))
            ps_s = ph.enter_context(tc.tile_pool(name="ps_s", bufs=2, space="PSUM"))
            ps_y = ph.enter_context(tc.tile_pool(name="ps_y", bufs=2, space="PSUM"))
            # shared by v-proj, qk-proj, normalization R, and proj outputs
            ps_r = ph.enter_context(tc.tile_pool(name="ps_r", bufs=2, space="PSUM"))

            # V projection: V[t, d] for all 8 heads at once (+bias via K=1 mm)
            with tc.tile_pool(name="wv", bufs=1) as p_wv:
                wv_sb = [p_wv.tile([P, QC], DT, tag=f"wv{j}", name=f"wv{j}") for j in range(CT)]
                for j in range(CT):
                    nc.sync.dma_start(wv_sb[j], wv[j * P:(j + 1) * P, :])
                for tt in range(SIN):
                    pv = ps_r.tile([P, QC], F32, tag="ps_r", name="pv")
                    for j in range(CT):
                        nc.tensor.matmul(
                            pv, lhsT=xt[j][:, tt * P:(tt + 1) * P],
                            rhs=wv_sb[j], start=(j == 0), stop=False)
                    nc.tensor.matmul(pv, lhsT=ones_row, rhs=bv_sb,
                                     start=False, stop=True)
                    nc.vector.tensor_copy(out=va[tt][:, :, 0:D], in_=pv)
                    nc.sync.dma_start(va[tt][:, :, D:D + 1], ones_in[:, 0:H][:, :, None])

            def qkproj(ch):
                wt = [p_wqk.tile([P, P], DT, tag="wqk", name="wqk") for _ in range(CT)]
                for j in range(CT):
                    nc.sync.dma_start(
                        wt[j], wqk[j * P:(j + 1) * P, ch * P:(ch + 1) * P])
                for tjc in range(TJN):
                    pq = ps_r.tile([P, 512], F32, tag="ps_r", name="pq")
                    for j in range(CT):
                        nc.tensor.matmul(
                            pq, lhsT=wt[j],
                            rhs=xt[j][:, tjc * 512:(tjc + 1) * 512],
                            start=(j == 0), stop=(j == CT - 1))
                    nc.vector.tensor_scalar_add(
                        out=qkt[ch][:, tjc * 512:(tjc + 1) * 512],
                        in0=pq, scalar1=bqk_sb[:, ch:ch + 1])

            def norm_batch(hp, tj, rcp_row, yun_a, yun_b):
                ts = slice(tj * 512, (tj + 1) * 512)
                for head, yun in ((0, yun_a), (1, yun_b)):
                    r = ps_r.tile([P, 512], F32, tag="ps_r", name="r")
                    nc.tensor.matmul(
                        r[0:D, :], lhsT=ones64f[D:D + 1, :],
                        rhs=rcp_row[D:D + 1, head * 512:(head + 1) * 512],
                        start=True, stop=True)
                    r_sb = p_rsb.tile([D, 512], F32, tag="rsb", name="r_sb")
                    nc.vector.tensor_copy(r_sb, r[0:D, :])
                    if head == 0:
                        nc.vector.tensor_mul(ysb[hp][0:D, ts], yun, r_sb)
                    else:
                        ynb = p_yn.tile([D, 512], DT, tag="yn", name="ynb")
                        nc.vector.tensor_mul(ynb, yun, r_sb)
                        nc.sync.dma_start(ysb[hp][D:P, ts], ynb)

            def proj_tile(tt, co):
                po = ps_r.tile([P, 512], F32, tag="ps_r", name="po")
                for i in range(QC // P):
                    nc.tensor.matmul(
                        po, lhsT=ysb[i][:, tt * P:(tt + 1) * P],
                        rhs=wpt[i][:, co * 512:(co + 1) * 512],
                        start=(i == 0), stop=(i == QC // P - 1))
                ot = p_o.tile([P, 512], F32, tag="o", name="ot")
                if tt % 2 == 0:
                    nc.vector.tensor_copy(ot, po)
                else:
                    nc.scalar.copy(ot, po)
                nc.sync.dma_start(
                    out[tt * P:(tt + 1) * P, co * 512:(co + 1) * 512], ot)

            pending = []
            projq = []  # ready-to-run proj tiles, popped between si iterations
            sictr = 0
            for tj in range(TJN):
                for hp in range(4):  # head pairs (local heads 2hp, 2hp+1)
                    if tj == 0:
                        qkproj(hp)
                        qkproj(4 + hp)
                    if hp == 1 and tj >= 1:
                        projq += [(tt, co) for tt in range(4 * (tj - 1), 4 * tj)
                                  for co in range(C // 512)]
                    qt, kt = qkt[hp], qkt[4 + hp]
                    nsi = 4 * tj + 4
                    ya = ps_y.tile([D + 1, 512], F32, tag="ps_y")
                    yb = ps_y.tile([D + 1, 512], F32, tag="ps_y")
                    for si in range(nsi):
                        m = si - 4 * tj  # diagonal-band index (>=0 on diag)
                        o = max(m, 0) * P  # first valid column in this block
                        s = ps_s.tile([P, 1024], F32, tag="ps_s")
                        nc.tensor.matmul(
                            s[:, o:512], lhsT=kt[0:D, si * P:(si + 1) * P],
                            rhs=qt[0:D, tj * 512 + o:(tj + 1) * 512],
                            start=True, stop=True)
                        nc.tensor.matmul(
                            s[:, 512 + o:1024], lhsT=kt[D:P, si * P:(si + 1) * P],
                            rhs=qt[D:P, tj * 512 + o:(tj + 1) * 512],
                            start=True, stop=True)
                        pt = p_pt.tile([P, 1024], DT, tag="pt")
                        if m < 0:
                            nc.scalar.activation(pt, s, EXP, scale=0.125)
                        else:
                            # one strided call covers both heads' valid range
                            pt2 = pt.rearrange("p (h w) -> p h w", h=2)
                            s2 = s.rearrange("p (h w) -> p h w", h=2)
                            nc.scalar.activation(pt2[:, :, o:512], s2[:, :, o:512],
                                                 EXP, scale=0.125)
                            nc.vector.tensor_tensor(
                                pt2[:, :, o:o + P], pt2[:, :, o:o + P],
                                tri_sb[:, None, :].to_broadcast((P, 2, P)),
                                mybir.AluOpType.mult)
                        nc.tensor.matmul(
                            ya[:, o:512], lhsT=va[si][:, 2 * hp, :],
                            rhs=pt[:, o:512],
                            start=(si == 0), stop=(si == nsi - 1))
                        nc.tensor.matmul(
                            yb[:, o:512], lhsT=va[si][:, 2 * hp + 1, :],
                            rhs=pt[:, 512 + o:1024],
                            start=(si == 0), stop=(si == nsi - 1))
                        sictr += 1
                        if projq and sictr % 3 == 0:
                            proj_tile(*projq.pop(0))
                    # release Y fast: copy unnormalized Y and the sums row
                    yun_a = p_yun.tile([D, 512], F32, tag="yun", name="yun_a")
                    yun_b = p_yun.tile([D, 512], F32, tag="yun", name="yun_b")
                    nc.vector.tensor_copy(yun_a, ya[0:D, :])
                    nc.vector.tensor_copy(yun_b, yb[0:D, :])
                    sumr = p_sumr.tile([P, 1024], F32, tag="sumr", name="sumr")
                    nc.vector.tensor_copy(sumr[D:D + 1, 0:512], ya[D:D + 1, :])
                    nc.vector.tensor_copy(sumr[D:D + 1, 512:1024], yb[D:D + 1, :])
                    # lane-parallel reciprocal via a DRAM bounce to [128, 8]
                    sums_d = p_dn.tile([1, 1024], F32, tag="sums_d", name="sums_d")
                    nc.sync.dma_start(sums_d, sumr[D:D + 1, :])
                    scat = p_scat.tile([P, 8], F32, tag="scat", name="scat")
                    nc.sync.dma_start(scat, sums_d.rearrange("1 (a b) -> a b", a=P))
                    scatr = p_scat.tile([P, 8], mybir.dt.float32r, tag="scatr",
                                        name="scatr")
                    with nc.allow_low_precision(reason="elementwise recip"):
                        nc.vector.reciprocal(scatr, scat)
                    rcp_d = p_dn.tile([1, 1024], mybir.dt.float32r, tag="rcp_d",
                                      name="rcp_d")
                    nc.sync.dma_start(rcp_d.rearrange("1 (a b) -> a b", a=P), scatr)
                    rcp_row = p_rcpr.tile([P, 1024], mybir.dt.float32r,
                                          tag="rcpr", name="rcp_row")
                    nc.sync.dma_start(rcp_row[D:D + 1, :], rcp_d)
                    pending.append((hp, tj, rcp_row, yun_a, yun_b))
                    if len(pending) >= 2:
                        norm_batch(*pending.pop(0))
            while pending:
                norm_batch(*pending.pop(0))
            while projq:
                proj_tile(*projq.pop(0))
            for tt in range(4 * (TJN - 1), 4 * TJN):
                for co in range(C // 512):
                    proj_tile(tt, co)

    nc.compile()
    return nc


_PROG = None


def _get_prog():
    global _PROG
    if _PROG is None:
        _PROG = build_program()
    return _PROG


_LAST_RESULT = {}


def kernel(x, W_attn, b_attn, W_proj, b_proj):
    x = np.asarray(x, np.float32)
    W_attn = np.asarray(W_attn, np.float32)
    b_attn = np.asarray(b_attn, np.float32)
    W_proj = np.asarray(W_proj, np.float32)
    b_proj = np.asarray(b_proj, np.float32)
    B = x.shape[0]
    nc = _get_prog()
    f = np.arange(P)[None, :]
    p = np.arange(P)[:, None]
    tri = (f >= p).astype(NPDT)
    cvt = lambda a: np.ascontiguousarray(a).astype(NPDT)
    in_maps = []
    for c in range(2 * B):
        b, hh = divmod(c, 2)
        sl = slice(hh * QC, hh * QC + QC)
        in_maps.append({
            "xT": cvt(x[b].T),
            "wqk": cvt(np.concatenate(
                [W_attn[:, sl], W_attn[:, C + hh * QC:C + hh * QC + QC]], axis=1)),
            "bqk": np.ascontiguousarray(np.concatenate(
                [b_attn[sl], b_attn[C + hh * QC:C + hh * QC + QC]])),
            "wv": cvt(W_attn[:, 2 * C + hh * QC:2 * C + hh * QC + QC]),
            "bv": cvt(b_attn[2 * C + hh * QC:2 * C + hh * QC + QC]),
            "wp": cvt(W_proj[hh * QC:hh * QC + QC, :]),
            "trimask": tri,
            "ones": np.ones((P, P), NPDT),
            "onesf": np.ones((P, D), np.float32),
        })
    res = run_bass_kernel_spmd(nc, in_maps, list(range(2 * B)), trace=TRACE)
    _LAST_RESULT["res"] = res
    out = np.empty((B, T, C), np.float32)
    for b in range(B):
        out[b] = res.results[2 * b]["out"] + res.results[2 * b + 1]["out"] + b_proj
    return out
